# revision 15
# baseline (speedup 1.0000x reference)
"""Per-domain batch normalization (BaseDomainBatchNorm) on 8 Trainium2 NeuronCores.

Math (reference):
    cnt[j]   = #{n : d[n] == j}            (clamped to >= 1)
    mean[j]  = sum_{d[n]==j} X[n] / cnt[j]
    var[j]   = sum_{d[n]==j} X[n]^2 / cnt[j] - mean[j]^2
    inv[j]   = rsqrt(var[j] + 1e-5)
    Y[n]     = X[n] * A[d[n]] + B[d[n]],  A = inv*gamma, B = beta - mean*A

Sharding: rows split 8192 per core; per-domain partial stats (sum/sumsq/cnt)
are AllReduce'd across the 8 cores; each core normalizes its own rows.

V2 design (vs the fp32 baseline):
  - X is loaded ONCE as bf16 via SWDGE cast-DMA in 8x 2MB transfers
    (fp32 HBM -> bf16 SBUF); no per-chunk f32->bf16 DVE casts.
  - stats per chunk: psum_s += onehot.T @ x_bf ; psum_q += onehot.T @ xsq
    (xsq alternates ACT Square / DVE mul to split engine load).
  - a tiny warm-up AllReduce issues at t~0 so the ~40us first-collective
    setup cost overlaps the X loads; the real stats AllReduce then runs
    at its marginal latency.
  - phase 2 per super-chunk (2 chunks, [128,2048] psum):
    4 gather matmuls produce [A(c0)|A(c1)|B(c0)|B(c1)] in one psum tile;
    one fp32->bf16 evacuation (ACT for most supers, DVE for the rest to
    balance); FMA on DVE in bf16 at 2x rate; result staged in bf16 and
    cast-stored (bf16 SBUF -> fp32 HBM) in 2MB transfers.
HBM traffic is the roofline minimum: read X once, write Y once.
"""

import numpy as np

N = 65536
C = 512
D = 16
NCORES = 8
SHARD = N // NCORES          # 8192 rows per core
P = 128                      # partitions
CHUNKS = SHARD // P          # 64 chunks of 128 rows
SUPERS = CHUNKS // 2         # 32 super-chunks
GROUPS = CHUNKS // 8         # 8 groups of 8 chunks (2MB DMA granularity)
EPS = 1e-5

_CACHE = {}


def _build_program():
    import concourse.bacc as bacc
    import concourse.tile as tile
    from concourse import mybir

    f32 = mybir.dt.float32
    bf16 = mybir.dt.bfloat16
    i32 = mybir.dt.int32
    Alu = mybir.AluOpType
    Act = mybir.ActivationFunctionType

    nc = bacc.Bacc("TRN2", target_bir_lowering=False, debug=False,
                   num_devices=NCORES)

    X_d = nc.dram_tensor("X", [SHARD, C], f32, kind="ExternalInput")
    d_d = nc.dram_tensor("d", [SHARD], i32, kind="ExternalInput")
    g_d = nc.dram_tensor("gamma", [D, C], f32, kind="ExternalInput")
    b_d = nc.dram_tensor("beta", [D, C], f32, kind="ExternalInput")
    Y_d = nc.dram_tensor("Y", [SHARD, C], f32, kind="ExternalOutput")

    cc_in = nc.dram_tensor("cc_in", [D, 2 * C + 1], f32)
    cc_out = nc.dram_tensor("cc_out", [D, 2 * C + 1], f32, addr_space="Shared")

    # partition p owns rows [p*64, (p+1)*64)
    Xv = X_d.ap().rearrange("(p n) c -> p n c", p=P)   # [128, 64, 512]
    Yv = Y_d.ap().rearrange("(p n) c -> p n c", p=P)

    DB = 1024  # d-broadcast strip width

    with tile.TileContext(nc) as tc:
        with (
            tc.tile_pool(name="const", bufs=1) as cpool,
            tc.tile_pool(name="x", bufs=GROUPS) as xpool,
            tc.tile_pool(name="sq", bufs=3) as sqpool,
            tc.tile_pool(name="oh", bufs=1) as ohpool,
            tc.tile_pool(name="small", bufs=1) as spool,
            tc.tile_pool(name="scr", bufs=2) as scrpool,
            tc.tile_pool(name="dbc", bufs=1) as dbcpool,
            tc.tile_pool(name="ab", bufs=3) as abpool,
            tc.tile_pool(name="y", bufs=2) as ypool,
        ):
            # ---- constants ----
            iota_rep = cpool.tile([P, CHUNKS, D], bf16)
            nc.gpsimd.iota(iota_rep[:], pattern=[[0, CHUNKS], [1, D]], base=0,
                           channel_multiplier=0,
                           allow_small_or_imprecise_dtypes=True)
            iota_i = cpool.tile([D, 1], i32)
            nc.gpsimd.iota(iota_i[:], pattern=[[0, 1]], base=0,
                           channel_multiplier=1)
            iota_col32 = cpool.tile([D, 1], f32)
            nc.vector.tensor_copy(iota_col32[:], iota_i[:])
            ones_col = cpool.tile([P, 1], bf16)
            nc.vector.memset(ones_col[:], 1.0)

            # ---- d in chunk layout and one-hot [128, 64, 16] ----
            d_pn = cpool.tile([P, CHUNKS], i32)
            nc.sync.dma_start(d_pn[:], d_d.ap().rearrange("(p n) -> p n", p=P))
            d_f = cpool.tile([P, CHUNKS], bf16)
            nc.vector.tensor_copy(d_f[:], d_pn[:])
            onehot = ohpool.tile([P, CHUNKS, D], bf16)
            nc.vector.tensor_tensor(
                onehot[:], iota_rep[:],
                d_f[:].unsqueeze(-1).broadcast_to([P, CHUNKS, D]),
                Alu.is_equal)

            # ---- transposed one-hot [16, 8192]: one broadcast cast-DMA of d
            # to 16 partitions, then a single is_equal; K=16 gathers need
            # no zero-padding ----
            d_bc = dbcpool.tile([D, SHARD], bf16)
            src = d_d.ap().rearrange("(a n) -> a n", a=1).partition_broadcast(D)
            nc.gpsimd.dma_start(d_bc[:], src)
            onehotT = ohpool.tile([D, SHARD], bf16)
            nc.vector.tensor_scalar(onehotT[:], d_bc[:], iota_col32[:], None,
                                    Alu.is_equal)

            # ---- X loads on BOTH DMA queues (~390 GB/s combined):
            # even groups: plain fp32 on sync (HWDGE) + one DVE cast
            # odd groups:  fp32->bf16 cast-DMA on gpsimd (SWDGE) ----
            xs = []
            for g in range(GROUPS):
                xt = xpool.tile([P, 8 * C], bf16)
                xs.append(xt)
            for g in range(GROUPS):
                if g % 2 == 0:
                    xf = scrpool.tile([P, 8 * C], f32, tag="xf")
                    nc.sync.dma_start(
                        xf[:].rearrange("p (n c) -> p n c", c=C),
                        Xv[:, 8 * g:8 * g + 8, :])
                    nc.vector.tensor_copy(xs[g][:], xf[:])
                else:
                    nc.gpsimd.dma_start(
                        xs[g][:].rearrange("p (n c) -> p n c", c=C),
                        Xv[:, 8 * g:8 * g + 8, :])

            # ---- AB2 table [16, 1024] = [A | B] per domain ----
            AB2 = spool.tile([D, 2 * C], bf16, tag="AB2")

            # ---- gamma/beta early loads ----
            gam = spool.tile([D, C], f32, tag="gam")
            nc.scalar.dma_start(gam[:], g_d[:])
            bet = spool.tile([D, C], f32, tag="bet")
            nc.scalar.dma_start(bet[:], b_d[:])

            # ---- phase 1: per-core partial stats ----
            stats = spool.tile([D, 2 * C + 1], f32, tag="stats")
            with tc.tile_pool(name="ps1", bufs=1, space="PSUM") as ps1:
                psum_s = ps1.tile([D, C], f32)
                psum_q = ps1.tile([D, C], f32)
                psum_c = ps1.tile([D, 1], f32)
                for i in range(CHUNKS):
                    g, k = divmod(i, 8)
                    xsl = xs[g][:, k * C:(k + 1) * C]
                    xsq = sqpool.tile([P, C], bf16, tag="xsq")
                    if i % 2 == 0:
                        nc.scalar.activation(xsq[:], xsl, Act.Square)
                    else:
                        nc.vector.tensor_mul(xsq[:], xsl, xsl)
                    oh = onehot[:, i, :]
                    st, sp = (i == 0), (i == CHUNKS - 1)
                    nc.tensor.matmul(psum_s[:], oh, xsl,
                                     start=st, stop=sp)
                    nc.tensor.matmul(psum_q[:], oh, xsq[:],
                                     start=st, stop=sp)

                # counts: reduce one-hot over chunks, then one matmul
                rowcnt = spool.tile([P, D], f32, tag="rowcnt")
                nc.vector.tensor_reduce(
                    rowcnt[:], onehot[:].rearrange("p n d -> p d n"),
                    mybir.AxisListType.X, Alu.add)
                rowcnt_bf = spool.tile([P, D], bf16, tag="rowcnt_bf")
                nc.vector.tensor_copy(rowcnt_bf[:], rowcnt[:])
                nc.tensor.matmul(psum_c[:], rowcnt_bf[:], ones_col[:],
                                 start=True, stop=True)

                nc.vector.tensor_copy(stats[:, 0:C], psum_s[:])
                nc.vector.tensor_copy(stats[:, C:2 * C], psum_q[:])
                nc.vector.tensor_copy(stats[:, 2 * C:2 * C + 1], psum_c[:])

                # keep the PE HAM clock-gate warm across the all-reduce stall
                warm = ps1.tile([P, C], f32)
                iflat = iota_rep[:].rearrange("p n d -> p (n d)")
                for _ in range(14):
                    nc.tensor.matmul(warm[:], iflat[:, 0:P],
                                     iflat[:, 0:C],
                                     start=True, stop=True,
                                     skip_group_check=True)

            # ---- all-reduce partial stats across the 8 cores ----
            nc.sync.dma_start(cc_in[:], stats[:])
            nc.gpsimd.collective_compute(
                "AllReduce", Alu.add,
                replica_groups=[list(range(NCORES))],
                ins=[cc_in[:]], outs=[cc_out[:]])
            red = spool.tile([D, 2 * C + 1], f32, tag="red")
            nc.sync.dma_start(red[:], cc_out[:])

            # ---- finalize (reciprocal-free):
            #   rsq = rsqrt(q*cnt - s^2 + eps*cnt^2)
            #   A   = gamma * cnt * rsq
            #   B   = beta - s * gamma * rsq
            # (identical to mean/var/rsqrt form for cnt >= 1)
            cntc = spool.tile([D, 1], f32, tag="cntc")
            nc.vector.tensor_scalar_max(cntc[:], red[:, 2 * C:2 * C + 1], 1.0)
            c2e = spool.tile([D, 1], f32, tag="c2e")
            nc.vector.tensor_tensor(c2e[:], cntc[:], cntc[:], Alu.mult)
            nc.vector.tensor_scalar_mul(c2e[:], c2e[:], EPS)
            t1 = scrpool.tile([D, C], f32, tag="scr")
            nc.vector.tensor_scalar_mul(t1[:], red[:, C:2 * C], cntc[:])
            t2 = spool.tile([D, C], f32, tag="t2")
            nc.vector.scalar_tensor_tensor(t2[:], red[:, 0:C], -1.0,
                                           red[:, 0:C], Alu.mult, Alu.mult)
            nc.vector.tensor_add(t1[:], t1[:], t2[:])   # q*cnt - s^2
            sd = scrpool.tile([D, C], f32, tag="scr")
            nc.scalar.activation(sd[:], t1[:], Act.Sqrt, bias=c2e[:])
            rsq = spool.tile([D, C], f32, tag="rsq")
            nc.vector.reciprocal(rsq[:], sd[:])
            ag = spool.tile([D, C], f32, tag="ag")
            nc.vector.tensor_mul(ag[:], gam[:], rsq[:])       # gamma*rsq
            a_t = spool.tile([D, C], f32, tag="a_t")
            nc.vector.tensor_scalar_mul(a_t[:], ag[:], cntc[:])
            b_t = spool.tile([D, C], f32, tag="b_t")
            nc.vector.scalar_tensor_tensor(b_t[:], red[:, 0:C], -1.0, ag[:],
                                           Alu.mult, Alu.mult)   # -s*gamma*rsq
            nc.vector.tensor_add(b_t[:], bet[:], b_t[:])

            nc.vector.tensor_copy(AB2[:, 0:C], a_t[:])
            nc.vector.tensor_copy(AB2[:, C:2 * C], b_t[:])

            # ---- phase 2: gather A/B rows (K=16 matmuls), FMA in bf16,
            # cast-store.  Two evac schemes to balance ACT vs DVE:
            #   A: ACT evacs [A|B] (FD=2048); DVE mul+add in bf16 (2x)
            #   B: ACT evacs A only (FD=1024); DVE mul bf16 + add from
            #      PSUM fp32 (1x)
            ohTv = onehotT[:].rearrange("k (p i) -> k i p", i=CHUNKS)
            SCHEME_B = {1, 3, 5, 7, 9, 11, 13}
            with tc.tile_pool(name="ps2", bufs=2, space="PSUM") as ps2:
                for g in range(GROUPS):
                    yb = ypool.tile([P, 8 * C], bf16)
                    for s in range(4 * g, 4 * g + 4):
                        q = s % 4
                        pab = ps2.tile([P, 4 * C], f32)
                        for k in range(2):
                            lt = ohTv[:, 2 * s + k, :]
                            nc.tensor.matmul(pab[:, k * C:(k + 1) * C],
                                             lt, AB2[:, 0:C],
                                             start=True, stop=True)
                            nc.tensor.matmul(pab[:, (2 + k) * C:(3 + k) * C],
                                             lt, AB2[:, C:2 * C],
                                             start=True, stop=True)
                        ysl = yb[:, q * 2 * C:(q + 1) * 2 * C]
                        xsl = xs[g][:, q * 2 * C:(q + 1) * 2 * C]
                        if (s % 16) in SCHEME_B:
                            a_sb = abpool.tile([P, 2 * C], bf16, tag="a_sb")
                            nc.scalar.activation(a_sb[:], pab[:, 0:2 * C],
                                                 Act.Copy)
                            nc.vector.tensor_mul(ysl, xsl, a_sb[:])
                            nc.vector.tensor_add(ysl, ysl,
                                                 pab[:, 2 * C:4 * C])
                        else:
                            ab = abpool.tile([P, 4 * C], bf16, tag="ab")
                            nc.scalar.activation(ab[:], pab[:], Act.Copy)
                            nc.vector.tensor_mul(ysl, xsl, ab[:, 0:2 * C])
                            nc.vector.tensor_add(ysl, ysl, ab[:, 2 * C:4 * C])
                    nc.gpsimd.dma_start(
                        Yv[:, 8 * g:8 * g + 8, :],
                        yb[:].rearrange("p (n c) -> p n c", c=C))

    nc.compile()
    return nc


def _get_program():
    if "nc" not in _CACHE:
        _CACHE["nc"] = _build_program()
    return _CACHE["nc"]


def kernel(X, d, parameter_t, fm_mean, gamma, beta):
    from concourse.bass_utils import run_bass_kernel_spmd

    X = np.ascontiguousarray(np.asarray(X), dtype=np.float32)
    d = np.ascontiguousarray(np.asarray(d), dtype=np.int32)
    gamma = np.ascontiguousarray(np.asarray(gamma), dtype=np.float32)
    beta = np.ascontiguousarray(np.asarray(beta), dtype=np.float32)

    nc = _get_program()
    in_maps = [
        {
            "X": X[c * SHARD:(c + 1) * SHARD],
            "d": d[c * SHARD:(c + 1) * SHARD],
            "gamma": gamma,
            "beta": beta,
        }
        for c in range(NCORES)
    ]
    res = run_bass_kernel_spmd(nc, in_maps, core_ids=list(range(NCORES)))
    out = np.concatenate([res.results[c]["Y"] for c in range(NCORES)], axis=0)
    return out.astype(np.float32, copy=False)


# revision 21
# speedup vs baseline: 1.0760x; 1.0760x over previous
"""Per-domain batch normalization (BaseDomainBatchNorm) on 8 Trainium2 NeuronCores.

Math (reference):
    cnt[j]   = #{n : d[n] == j}            (clamped to >= 1)
    mean[j]  = sum_{d[n]==j} X[n] / cnt[j]
    var[j]   = sum_{d[n]==j} X[n]^2 / cnt[j] - mean[j]^2
    inv[j]   = rsqrt(var[j] + 1e-5)
    Y[n]     = X[n] * A[d[n]] + B[d[n]],  A = inv*gamma, B = beta - mean*A

Sharding: rows split 8192 per core; per-domain partial stats (sum/sumsq/cnt)
are AllReduce'd across the 8 cores; each core normalizes its own rows.

V2 design (vs the fp32 baseline):
  - X is loaded ONCE as bf16 via SWDGE cast-DMA in 8x 2MB transfers
    (fp32 HBM -> bf16 SBUF); no per-chunk f32->bf16 DVE casts.
  - stats per chunk: psum_s += onehot.T @ x_bf ; psum_q += onehot.T @ xsq
    (xsq alternates ACT Square / DVE mul to split engine load).
  - a tiny warm-up AllReduce issues at t~0 so the ~40us first-collective
    setup cost overlaps the X loads; the real stats AllReduce then runs
    at its marginal latency.
  - phase 2 per super-chunk (2 chunks, [128,2048] psum):
    4 gather matmuls produce [A(c0)|A(c1)|B(c0)|B(c1)] in one psum tile;
    one fp32->bf16 evacuation (ACT for most supers, DVE for the rest to
    balance); FMA on DVE in bf16 at 2x rate; result staged in bf16 and
    cast-stored (bf16 SBUF -> fp32 HBM) in 2MB transfers.
HBM traffic is the roofline minimum: read X once, write Y once.
"""

import numpy as np

N = 65536
C = 512
D = 16
NCORES = 8
SHARD = N // NCORES          # 8192 rows per core
P = 128                      # partitions
CHUNKS = SHARD // P          # 64 chunks of 128 rows
SUPERS = CHUNKS // 2         # 32 super-chunks
GROUPS = CHUNKS // 8         # 8 groups of 8 chunks (2MB DMA granularity)
EPS = 1e-5

_CACHE = {}


def _build_program():
    import concourse.bacc as bacc
    import concourse.tile as tile
    from concourse import mybir

    f32 = mybir.dt.float32
    bf16 = mybir.dt.bfloat16
    i32 = mybir.dt.int32
    Alu = mybir.AluOpType
    Act = mybir.ActivationFunctionType

    nc = bacc.Bacc("TRN2", target_bir_lowering=False, debug=False,
                   num_devices=NCORES)

    X_d = nc.dram_tensor("X", [SHARD, C], f32, kind="ExternalInput")
    d_d = nc.dram_tensor("d", [SHARD], i32, kind="ExternalInput")
    g_d = nc.dram_tensor("gamma", [D, C], f32, kind="ExternalInput")
    b_d = nc.dram_tensor("beta", [D, C], f32, kind="ExternalInput")
    Y_d = nc.dram_tensor("Y", [SHARD, C], f32, kind="ExternalOutput")

    cc_in = nc.dram_tensor("cc_in", [D, 2 * C + 1], f32)
    cc_out = nc.dram_tensor("cc_out", [D, 2 * C + 1], f32, addr_space="Shared")

    # partition p owns rows [p*64, (p+1)*64)
    Xv = X_d.ap().rearrange("(p n) c -> p n c", p=P)   # [128, 64, 512]
    Yv = Y_d.ap().rearrange("(p n) c -> p n c", p=P)

    DB = 1024  # d-broadcast strip width

    with tile.TileContext(nc) as tc:
        with (
            tc.tile_pool(name="const", bufs=1) as cpool,
            tc.tile_pool(name="x", bufs=GROUPS) as xpool,
            tc.tile_pool(name="sq", bufs=3) as sqpool,
            tc.tile_pool(name="oh", bufs=1) as ohpool,
            tc.tile_pool(name="small", bufs=1) as spool,
            tc.tile_pool(name="scr", bufs=2) as scrpool,
            tc.tile_pool(name="dbc", bufs=2) as dbcpool,
            tc.tile_pool(name="ab", bufs=3) as abpool,
            tc.tile_pool(name="y", bufs=2) as ypool,
        ):
            # ---- constants ----
            iota_rep = cpool.tile([P, CHUNKS, D], bf16)
            nc.gpsimd.iota(iota_rep[:], pattern=[[0, CHUNKS], [1, D]], base=0,
                           channel_multiplier=0,
                           allow_small_or_imprecise_dtypes=True)
            iota_i = cpool.tile([2 * D, 1], i32)
            nc.gpsimd.iota(iota_i[:], pattern=[[0, 1]], base=0,
                           channel_multiplier=1)
            nc.vector.tensor_scalar(iota_i[:], iota_i[:], D - 1, None,
                                    Alu.bitwise_and)
            iota_col32 = cpool.tile([2 * D, 1], f32)
            nc.vector.tensor_copy(iota_col32[:], iota_i[:])
            ones_col = cpool.tile([P, 1], bf16)
            nc.vector.memset(ones_col[:], 1.0)

            # ---- d in chunk layout and one-hot [128, 64, 16] ----
            d_pn = cpool.tile([P, CHUNKS], i32)
            nc.sync.dma_start(d_pn[:], d_d.ap().rearrange("(p n) -> p n", p=P))
            d_f = cpool.tile([P, CHUNKS], bf16)
            nc.vector.tensor_copy(d_f[:], d_pn[:])
            onehot = ohpool.tile([P, CHUNKS, D], bf16)
            nc.vector.tensor_tensor(
                onehot[:], iota_rep[:],
                d_f[:].unsqueeze(-1).broadcast_to([P, CHUNKS, D]),
                Alu.is_equal)

            # ---- transposed one-hot [128, 8192] (K=128 keeps the PE HAM
            # ramped); rows 0:32 real, 32:128 defined-garbage against a
            # zero-padded table.  All build DMAs ride the scalar (HWDGE)
            # queue so the gpsimd queue is free for X cast-loads. ----
            onehotT = ohpool.tile([P, SHARD], bf16)
            for h in range(SHARD // DB):
                d_bc = dbcpool.tile([2 * D, DB], i32)
                src = d_d.ap()[h * DB:(h + 1) * DB]
                src = src.rearrange("(a n) -> a n", a=1).partition_broadcast(2 * D)
                nc.scalar.dma_start(d_bc[:], src)
                nc.vector.tensor_scalar(onehotT[0:2 * D, h * DB:(h + 1) * DB],
                                        d_bc[:], iota_col32[:], None,
                                        Alu.is_equal)
            for pb in range(2 * D, P, 2 * D):
                nc.scalar.dma_start(onehotT[pb:pb + 2 * D, :],
                                    onehotT[0:2 * D, :])

            # ---- X loads on BOTH DMA queues (~390 GB/s combined):
            # even groups: plain fp32 on sync (HWDGE) + one DVE cast
            # odd groups:  fp32->bf16 cast-DMA on gpsimd (SWDGE) ----
            xs = []
            for g in range(GROUPS):
                xt = xpool.tile([P, 8 * C], bf16)
                xs.append(xt)
            for g in range(GROUPS):
                if g % 2 == 0:
                    xf = scrpool.tile([P, 8 * C], f32, tag="xf")
                    nc.sync.dma_start(
                        xf[:].rearrange("p (n c) -> p n c", c=C),
                        Xv[:, 8 * g:8 * g + 8, :])
                    nc.vector.tensor_copy(xs[g][:], xf[:])
                else:
                    nc.gpsimd.dma_start(
                        xs[g][:].rearrange("p (n c) -> p n c", c=C),
                        Xv[:, 8 * g:8 * g + 8, :])

            # ---- AB2 table [128, 1024]: rows 0:16 = [A | B], rest zero ----
            AB2 = spool.tile([P, 2 * C], bf16, tag="AB2")
            for pb in range(2 * D, P, 2 * D):
                nc.vector.memset(AB2[pb:pb + 2 * D, :], 0.0)
            nc.scalar.dma_start(AB2[D:2 * D, :], AB2[2 * D:2 * D + D, :])

            # ---- gamma/beta early loads ----
            gam = spool.tile([D, C], f32, tag="gam")
            nc.scalar.dma_start(gam[:], g_d[:])
            bet = spool.tile([D, C], f32, tag="bet")
            nc.scalar.dma_start(bet[:], b_d[:])

            # ---- phase 1: per-core partial stats ----
            stats = spool.tile([D, 2 * C + 1], f32, tag="stats")
            with tc.tile_pool(name="ps1", bufs=1, space="PSUM") as ps1:
                psum_s = ps1.tile([D, C], f32)
                psum_q = ps1.tile([D, C], f32)
                psum_c = ps1.tile([D, 1], f32)
                for i in range(CHUNKS):
                    g, k = divmod(i, 8)
                    xsl = xs[g][:, k * C:(k + 1) * C]
                    xsq = sqpool.tile([P, C], bf16, tag="xsq")
                    if i % 2 == 0:
                        nc.scalar.activation(xsq[:], xsl, Act.Square)
                    else:
                        nc.vector.tensor_mul(xsq[:], xsl, xsl)
                    oh = onehot[:, i, :]
                    st, sp = (i == 0), (i == CHUNKS - 1)
                    nc.tensor.matmul(psum_s[:], oh, xsl,
                                     start=st, stop=sp)
                    nc.tensor.matmul(psum_q[:], oh, xsq[:],
                                     start=st, stop=sp)

                # counts: reduce one-hot over chunks, then one matmul
                rowcnt = spool.tile([P, D], f32, tag="rowcnt")
                nc.vector.tensor_reduce(
                    rowcnt[:], onehot[:].rearrange("p n d -> p d n"),
                    mybir.AxisListType.X, Alu.add)
                rowcnt_bf = spool.tile([P, D], bf16, tag="rowcnt_bf")
                nc.vector.tensor_copy(rowcnt_bf[:], rowcnt[:])
                nc.tensor.matmul(psum_c[:], rowcnt_bf[:], ones_col[:],
                                 start=True, stop=True)

                nc.vector.tensor_copy(stats[:, 0:C], psum_s[:])
                nc.vector.tensor_copy(stats[:, C:2 * C], psum_q[:])
                nc.vector.tensor_copy(stats[:, 2 * C:2 * C + 1], psum_c[:])

                # keep the PE HAM clock-gate warm across the all-reduce stall
                warm = ps1.tile([P, C], f32)
                iflat = iota_rep[:].rearrange("p n d -> p (n d)")
                for _ in range(32):
                    nc.tensor.matmul(warm[:], iflat[:, 0:P],
                                     iflat[:, 0:C],
                                     start=True, stop=True,
                                     skip_group_check=True)

            # ---- all-reduce partial stats across the 8 cores ----
            nc.sync.dma_start(cc_in[:], stats[:])
            nc.gpsimd.collective_compute(
                "AllReduce", Alu.add,
                replica_groups=[list(range(NCORES))],
                ins=[cc_in[:]], outs=[cc_out[:]])
            red = spool.tile([D, 2 * C + 1], f32, tag="red")
            nc.sync.dma_start(red[:], cc_out[:])

            # ---- finalize (reciprocal-free):
            #   rsq = rsqrt(q*cnt - s^2 + eps*cnt^2)
            #   A   = gamma * cnt * rsq
            #   B   = beta - s * gamma * rsq
            # (identical to mean/var/rsqrt form for cnt >= 1)
            cntc = spool.tile([D, 1], f32, tag="cntc")
            nc.vector.tensor_scalar_max(cntc[:], red[:, 2 * C:2 * C + 1], 1.0)
            c2e = spool.tile([D, 1], f32, tag="c2e")
            nc.vector.tensor_tensor(c2e[:], cntc[:], cntc[:], Alu.mult)
            nc.vector.tensor_scalar_mul(c2e[:], c2e[:], EPS)
            t1 = scrpool.tile([D, C], f32, tag="scr")
            nc.vector.tensor_scalar_mul(t1[:], red[:, C:2 * C], cntc[:])
            t2 = spool.tile([D, C], f32, tag="t2")
            nc.vector.scalar_tensor_tensor(t2[:], red[:, 0:C], -1.0,
                                           red[:, 0:C], Alu.mult, Alu.mult)
            nc.vector.tensor_add(t1[:], t1[:], t2[:])   # q*cnt - s^2
            sd = scrpool.tile([D, C], f32, tag="scr")
            nc.scalar.activation(sd[:], t1[:], Act.Sqrt, bias=c2e[:])
            rsq = spool.tile([D, C], f32, tag="rsq")
            nc.vector.reciprocal(rsq[:], sd[:])
            ag = spool.tile([D, C], f32, tag="ag")
            nc.vector.tensor_mul(ag[:], gam[:], rsq[:])       # gamma*rsq
            a_t = spool.tile([D, C], f32, tag="a_t")
            nc.vector.tensor_scalar_mul(a_t[:], ag[:], cntc[:])
            b_t = spool.tile([D, C], f32, tag="b_t")
            nc.vector.scalar_tensor_tensor(b_t[:], red[:, 0:C], -1.0, ag[:],
                                           Alu.mult, Alu.mult)   # -s*gamma*rsq
            nc.vector.tensor_add(b_t[:], bet[:], b_t[:])

            nc.vector.tensor_copy(AB2[0:D, 0:C], a_t[:])
            nc.vector.tensor_copy(AB2[0:D, C:2 * C], b_t[:])

            # ---- phase 2: gather A/B rows (K=16 matmuls), FMA in bf16,
            # cast-store.  Two evac schemes to balance ACT vs DVE:
            #   A: ACT evacs [A|B] (FD=2048); DVE mul+add in bf16 (2x)
            #   B: ACT evacs A only (FD=1024); DVE mul bf16 + add from
            #      PSUM fp32 (1x)
            ohTv = onehotT[:].rearrange("k (p i) -> k i p", i=CHUNKS)
            SCHEME_B = {1, 3, 5, 7, 9, 11, 13}
            with tc.tile_pool(name="ps2", bufs=2, space="PSUM") as ps2:
                for g in range(GROUPS):
                    yb = ypool.tile([P, 8 * C], bf16)
                    for s in range(4 * g, 4 * g + 4):
                        q = s % 4
                        pab = ps2.tile([P, 4 * C], f32)
                        for k in range(2):
                            lt = ohTv[:, 2 * s + k, :]
                            nc.tensor.matmul(pab[:, k * C:(k + 1) * C],
                                             lt, AB2[:, 0:C],
                                             start=True, stop=True)
                            nc.tensor.matmul(pab[:, (2 + k) * C:(3 + k) * C],
                                             lt, AB2[:, C:2 * C],
                                             start=True, stop=True)
                        ysl = yb[:, q * 2 * C:(q + 1) * 2 * C]
                        xsl = xs[g][:, q * 2 * C:(q + 1) * 2 * C]
                        if (s % 16) in SCHEME_B:
                            a_sb = abpool.tile([P, 2 * C], bf16, tag="a_sb")
                            nc.scalar.activation(a_sb[:], pab[:, 0:2 * C],
                                                 Act.Copy)
                            nc.vector.tensor_mul(ysl, xsl, a_sb[:])
                            nc.vector.tensor_add(ysl, ysl,
                                                 pab[:, 2 * C:4 * C])
                        else:
                            ab = abpool.tile([P, 4 * C], bf16, tag="ab")
                            nc.scalar.activation(ab[:], pab[:], Act.Copy)
                            nc.vector.tensor_mul(ysl, xsl, ab[:, 0:2 * C])
                            nc.vector.tensor_add(ysl, ysl, ab[:, 2 * C:4 * C])
                    nc.gpsimd.dma_start(
                        Yv[:, 8 * g:8 * g + 8, :],
                        yb[:].rearrange("p (n c) -> p n c", c=C))

    nc.compile()
    return nc


def _get_program():
    if "nc" not in _CACHE:
        _CACHE["nc"] = _build_program()
    return _CACHE["nc"]


def kernel(X, d, parameter_t, fm_mean, gamma, beta):
    from concourse.bass_utils import run_bass_kernel_spmd

    X = np.ascontiguousarray(np.asarray(X), dtype=np.float32)
    d = np.ascontiguousarray(np.asarray(d), dtype=np.int32)
    gamma = np.ascontiguousarray(np.asarray(gamma), dtype=np.float32)
    beta = np.ascontiguousarray(np.asarray(beta), dtype=np.float32)

    nc = _get_program()
    in_maps = [
        {
            "X": X[c * SHARD:(c + 1) * SHARD],
            "d": d[c * SHARD:(c + 1) * SHARD],
            "gamma": gamma,
            "beta": beta,
        }
        for c in range(NCORES)
    ]
    res = run_bass_kernel_spmd(nc, in_maps, core_ids=list(range(NCORES)))
    out = np.concatenate([res.results[c]["Y"] for c in range(NCORES)], axis=0)
    return out.astype(np.float32, copy=False)


# revision 26
# speedup vs baseline: 1.1307x; 1.0508x over previous
"""Per-domain batch normalization (BaseDomainBatchNorm) on 8 Trainium2 NeuronCores.

Math (reference):
    cnt[j]   = #{n : d[n] == j}            (clamped to >= 1)
    mean[j]  = sum_{d[n]==j} X[n] / cnt[j]
    var[j]   = sum_{d[n]==j} X[n]^2 / cnt[j] - mean[j]^2
    inv[j]   = rsqrt(var[j] + 1e-5)
    Y[n]     = X[n] * A[d[n]] + B[d[n]],  A = inv*gamma, B = beta - mean*A

Sharding: rows split 8192 per core; per-domain partial stats (sum/sumsq/cnt)
are AllReduce'd across the 8 cores; each core normalizes its own rows.

V2 design (vs the fp32 baseline):
  - X is loaded ONCE as bf16 via SWDGE cast-DMA in 8x 2MB transfers
    (fp32 HBM -> bf16 SBUF); no per-chunk f32->bf16 DVE casts.
  - stats per chunk: psum_s += onehot.T @ x_bf ; psum_q += onehot.T @ xsq
    (xsq alternates ACT Square / DVE mul to split engine load).
  - a tiny warm-up AllReduce issues at t~0 so the ~40us first-collective
    setup cost overlaps the X loads; the real stats AllReduce then runs
    at its marginal latency.
  - phase 2 per super-chunk (2 chunks, [128,2048] psum):
    4 gather matmuls produce [A(c0)|A(c1)|B(c0)|B(c1)] in one psum tile;
    one fp32->bf16 evacuation (ACT for most supers, DVE for the rest to
    balance); FMA on DVE in bf16 at 2x rate; result staged in bf16 and
    cast-stored (bf16 SBUF -> fp32 HBM) in 2MB transfers.
HBM traffic is the roofline minimum: read X once, write Y once.
"""

import numpy as np

N = 65536
C = 512
D = 16
NCORES = 8
SHARD = N // NCORES          # 8192 rows per core
P = 128                      # partitions
CHUNKS = SHARD // P          # 64 chunks of 128 rows
SUPERS = CHUNKS // 2         # 32 super-chunks
GROUPS = CHUNKS // 8         # 8 groups of 8 chunks (2MB DMA granularity)
EPS = 1e-5

_CACHE = {}


def _build_program():
    import concourse.bacc as bacc
    import concourse.tile as tile
    from concourse import mybir

    f32 = mybir.dt.float32
    bf16 = mybir.dt.bfloat16
    i32 = mybir.dt.int32
    Alu = mybir.AluOpType
    Act = mybir.ActivationFunctionType

    nc = bacc.Bacc("TRN2", target_bir_lowering=False, debug=False,
                   num_devices=NCORES)

    X_d = nc.dram_tensor("X", [SHARD, C], f32, kind="ExternalInput")
    d_d = nc.dram_tensor("d", [SHARD], i32, kind="ExternalInput")
    g_d = nc.dram_tensor("gamma", [D, C], f32, kind="ExternalInput")
    b_d = nc.dram_tensor("beta", [D, C], f32, kind="ExternalInput")
    # host-provided compile-time-constant iotas (avoids gpsimd iota ops,
    # whose library load would delay the cast-load DMA queue)
    cr_d = nc.dram_tensor("cst_rep", [P, CHUNKS * D], bf16,
                          kind="ExternalInput")
    cc_d = nc.dram_tensor("cst_col", [D, 1], f32, kind="ExternalInput")
    Y_d = nc.dram_tensor("Y", [SHARD, C], f32, kind="ExternalOutput")

    cc_in = nc.dram_tensor("cc_in", [D, 2 * C + 1], f32)
    cc_out = nc.dram_tensor("cc_out", [D, 2 * C + 1], f32, addr_space="Shared")

    # partition p owns rows [p*64, (p+1)*64)
    Xv = X_d.ap().rearrange("(p n) c -> p n c", p=P)   # [128, 64, 512]
    Yv = Y_d.ap().rearrange("(p n) c -> p n c", p=P)

    DB = 1024  # d-broadcast strip width

    with tile.TileContext(nc) as tc:
        with (
            tc.tile_pool(name="const", bufs=1) as cpool,
            tc.tile_pool(name="x", bufs=GROUPS) as xpool,
            tc.tile_pool(name="sq", bufs=3) as sqpool,
            tc.tile_pool(name="oh", bufs=1) as ohpool,
            tc.tile_pool(name="small", bufs=1) as spool,
            tc.tile_pool(name="scr", bufs=2) as scrpool,
            tc.tile_pool(name="dbc", bufs=2) as dbcpool,
            tc.tile_pool(name="ab", bufs=3) as abpool,
            tc.tile_pool(name="y", bufs=2) as ypool,
        ):
            # ---- X loads issue first on BOTH DMA queues:
            # even groups: plain fp32 on sync (HWDGE) + one DVE cast
            # odd groups:  fp32->bf16 cast-DMA on gpsimd (SWDGE) ----
            d_pn = cpool.tile([P, CHUNKS], i32)
            nc.sync.dma_start(d_pn[:], d_d.ap().rearrange("(p n) -> p n", p=P))
            xs = []
            for g in range(GROUPS):
                xt = xpool.tile([P, 8 * C], bf16)
                xs.append(xt)
            for g in range(GROUPS):
                if g % 2 == 0:
                    xf = scrpool.tile([P, 8 * C], f32, tag="xf")
                    nc.sync.dma_start(
                        xf[:].rearrange("p (n c) -> p n c", c=C),
                        Xv[:, 8 * g:8 * g + 8, :])
                    nc.vector.tensor_copy(xs[g][:], xf[:])
                else:
                    nc.gpsimd.dma_start(
                        xs[g][:].rearrange("p (n c) -> p n c", c=C),
                        Xv[:, 8 * g:8 * g + 8, :])

            # ---- constants (scalar queue) ----
            iota_rep = cpool.tile([P, CHUNKS, D], bf16)
            nc.scalar.dma_start(
                iota_rep[:].rearrange("p n d -> p (n d)"), cr_d.ap())
            iota_col32 = cpool.tile([D, 1], f32)
            nc.scalar.dma_start(iota_col32[:], cc_d.ap())
            ones_col = cpool.tile([P, 1], bf16)
            nc.vector.memset(ones_col[:], 1.0)

            # ---- one-hot [128, 64, 16] in chunk layout ----
            d_f = cpool.tile([P, CHUNKS], bf16)
            nc.vector.tensor_copy(d_f[:], d_pn[:])
            onehot = ohpool.tile([P, CHUNKS, D], bf16)
            nc.vector.tensor_tensor(
                onehot[:], iota_rep[:],
                d_f[:].unsqueeze(-1).broadcast_to([P, CHUNKS, D]),
                Alu.is_equal)

            # ---- AB2 table [128, 1024]: rows 0:16 = [A | B], rest zero ----
            AB2 = spool.tile([P, 2 * C], bf16, tag="AB2")
            for pb in range(2 * D, P, 2 * D):
                nc.vector.memset(AB2[pb:pb + 2 * D, :], 0.0)
            nc.scalar.dma_start(AB2[D:2 * D, :], AB2[2 * D:2 * D + D, :])

            # ---- gamma/beta early loads ----
            gam = spool.tile([D, C], f32, tag="gam")
            nc.scalar.dma_start(gam[:], g_d[:])
            bet = spool.tile([D, C], f32, tag="bet")
            nc.scalar.dma_start(bet[:], b_d[:])

            # ---- phase 1: per-core partial stats ----
            stats = spool.tile([D, 2 * C + 1], f32, tag="stats")
            with tc.tile_pool(name="ps1", bufs=1, space="PSUM") as ps1:
                psum_s = ps1.tile([D, C], f32)
                psum_q = ps1.tile([D, C], f32)
                psum_c = ps1.tile([D, 1], f32)
                for i in range(CHUNKS):
                    g, k = divmod(i, 8)
                    xsl = xs[g][:, k * C:(k + 1) * C]
                    xsq = sqpool.tile([P, C], bf16, tag="xsq")
                    if i % 2 == 0:
                        nc.scalar.activation(xsq[:], xsl, Act.Square)
                    else:
                        nc.vector.tensor_mul(xsq[:], xsl, xsl)
                    oh = onehot[:, i, :]
                    st, sp = (i == 0), (i == CHUNKS - 1)
                    nc.tensor.matmul(psum_s[:], oh, xsl,
                                     start=st, stop=sp)
                    nc.tensor.matmul(psum_q[:], oh, xsq[:],
                                     start=st, stop=sp)

                # counts: reduce one-hot over chunks, then one matmul
                rowcnt = spool.tile([P, D], f32, tag="rowcnt")
                nc.vector.tensor_reduce(
                    rowcnt[:], onehot[:].rearrange("p n d -> p d n"),
                    mybir.AxisListType.X, Alu.add)
                rowcnt_bf = spool.tile([P, D], bf16, tag="rowcnt_bf")
                nc.vector.tensor_copy(rowcnt_bf[:], rowcnt[:])
                nc.tensor.matmul(psum_c[:], rowcnt_bf[:], ones_col[:],
                                 start=True, stop=True)

                nc.vector.tensor_copy(stats[:, 0:C], psum_s[:])
                nc.vector.tensor_copy(stats[:, C:2 * C], psum_q[:])
                nc.vector.tensor_copy(stats[:, 2 * C:2 * C + 1], psum_c[:])

                # keep the PE HAM clock-gate warm across the all-reduce stall
                warm = ps1.tile([P, C], f32)
                iflat = iota_rep[:].rearrange("p n d -> p (n d)")
                for _ in range(32):
                    nc.tensor.matmul(warm[:], iflat[:, 0:P],
                                     iflat[:, 0:C],
                                     start=True, stop=True,
                                     skip_group_check=True)

            # ---- transposed one-hot [128, 8192], built during the
            # all-reduce window: rows 0:16 via broadcast strips + is_equal,
            # then partition-doubling SBUF copies to fill 16:128 (their
            # table rows are zero).  The gpsimd queue is free of X loads
            # by now. ----
            onehotT = ohpool.tile([P, SHARD], bf16)
            for h in range(SHARD // DB):
                d_bc = dbcpool.tile([D, DB], i32)
                src = d_d.ap()[h * DB:(h + 1) * DB]
                src = src.rearrange("(a n) -> a n", a=1).partition_broadcast(D)
                nc.gpsimd.dma_start(d_bc[:], src)
                nc.vector.tensor_scalar(onehotT[0:D, h * DB:(h + 1) * DB],
                                        d_bc[:], iota_col32[:], None,
                                        Alu.is_equal)
            for pb in (D, 2 * D, 4 * D):
                nc.gpsimd.dma_start(onehotT[pb:2 * pb, :], onehotT[0:pb, :])

            # ---- all-reduce partial stats across the 8 cores ----
            nc.sync.dma_start(cc_in[:], stats[:])
            nc.gpsimd.collective_compute(
                "AllReduce", Alu.add,
                replica_groups=[list(range(NCORES))],
                ins=[cc_in[:]], outs=[cc_out[:]])
            red = spool.tile([D, 2 * C + 1], f32, tag="red")
            nc.sync.dma_start(red[:], cc_out[:])

            # ---- finalize (reciprocal-free):
            #   rsq = rsqrt(q*cnt - s^2 + eps*cnt^2)
            #   A   = gamma * cnt * rsq
            #   B   = beta - s * gamma * rsq
            # (identical to mean/var/rsqrt form for cnt >= 1)
            cntc = spool.tile([D, 1], f32, tag="cntc")
            nc.vector.tensor_scalar_max(cntc[:], red[:, 2 * C:2 * C + 1], 1.0)
            c2e = spool.tile([D, 1], f32, tag="c2e")
            nc.vector.tensor_tensor(c2e[:], cntc[:], cntc[:], Alu.mult)
            nc.vector.tensor_scalar_mul(c2e[:], c2e[:], EPS)
            t1 = scrpool.tile([D, C], f32, tag="scr")
            nc.vector.tensor_scalar_mul(t1[:], red[:, C:2 * C], cntc[:])
            t2 = spool.tile([D, C], f32, tag="t2")
            nc.vector.scalar_tensor_tensor(t2[:], red[:, 0:C], -1.0,
                                           red[:, 0:C], Alu.mult, Alu.mult)
            nc.vector.tensor_add(t1[:], t1[:], t2[:])   # q*cnt - s^2
            sd = scrpool.tile([D, C], f32, tag="scr")
            nc.scalar.activation(sd[:], t1[:], Act.Sqrt, bias=c2e[:])
            rsq = spool.tile([D, C], f32, tag="rsq")
            nc.vector.reciprocal(rsq[:], sd[:])
            ag = spool.tile([D, C], f32, tag="ag")
            nc.vector.tensor_mul(ag[:], gam[:], rsq[:])       # gamma*rsq
            a_t = spool.tile([D, C], f32, tag="a_t")
            nc.vector.tensor_scalar_mul(a_t[:], ag[:], cntc[:])
            b_t = spool.tile([D, C], f32, tag="b_t")
            nc.vector.scalar_tensor_tensor(b_t[:], red[:, 0:C], -1.0, ag[:],
                                           Alu.mult, Alu.mult)   # -s*gamma*rsq
            nc.vector.tensor_add(b_t[:], bet[:], b_t[:])

            nc.vector.tensor_copy(AB2[0:D, 0:C], a_t[:])
            nc.vector.tensor_copy(AB2[0:D, C:2 * C], b_t[:])

            # ---- phase 2: gather A/B rows (K=16 matmuls), FMA in bf16,
            # cast-store.  Two evac schemes to balance ACT vs DVE:
            #   A: ACT evacs [A|B] (FD=2048); DVE mul+add in bf16 (2x)
            #   B: ACT evacs A only (FD=1024); DVE mul bf16 + add from
            #      PSUM fp32 (1x)
            ohTv = onehotT[:].rearrange("k (p i) -> k i p", i=CHUNKS)
            SCHEME_B = {1, 3, 5, 7, 9, 11, 13}
            with tc.tile_pool(name="ps2", bufs=2, space="PSUM") as ps2:
                for g in range(GROUPS):
                    yb = ypool.tile([P, 8 * C], bf16)
                    for s in range(4 * g, 4 * g + 4):
                        q = s % 4
                        pab = ps2.tile([P, 4 * C], f32)
                        for k in range(2):
                            lt = ohTv[:, 2 * s + k, :]
                            nc.tensor.matmul(pab[:, k * C:(k + 1) * C],
                                             lt, AB2[:, 0:C],
                                             start=True, stop=True)
                            nc.tensor.matmul(pab[:, (2 + k) * C:(3 + k) * C],
                                             lt, AB2[:, C:2 * C],
                                             start=True, stop=True)
                        ysl = yb[:, q * 2 * C:(q + 1) * 2 * C]
                        xsl = xs[g][:, q * 2 * C:(q + 1) * 2 * C]
                        if (s % 16) in SCHEME_B:
                            a_sb = abpool.tile([P, 2 * C], bf16, tag="a_sb")
                            nc.scalar.activation(a_sb[:], pab[:, 0:2 * C],
                                                 Act.Copy)
                            nc.vector.tensor_mul(ysl, xsl, a_sb[:])
                            nc.vector.tensor_add(ysl, ysl,
                                                 pab[:, 2 * C:4 * C])
                        else:
                            ab = abpool.tile([P, 4 * C], bf16, tag="ab")
                            nc.scalar.activation(ab[:], pab[:], Act.Copy)
                            nc.vector.tensor_mul(ysl, xsl, ab[:, 0:2 * C])
                            nc.vector.tensor_add(ysl, ysl, ab[:, 2 * C:4 * C])
                    nc.gpsimd.dma_start(
                        Yv[:, 8 * g:8 * g + 8, :],
                        yb[:].rearrange("p (n c) -> p n c", c=C))

    nc.compile()
    return nc


def _get_program():
    if "nc" not in _CACHE:
        _CACHE["nc"] = _build_program()
    return _CACHE["nc"]


def _constants():
    if "cst" not in _CACHE:
        import ml_dtypes
        rep = np.tile(np.arange(D, dtype=np.float32), (P, CHUNKS))
        cst_rep = rep.astype(ml_dtypes.bfloat16)       # [P, CHUNKS*D]
        cst_col = np.arange(D, dtype=np.float32).reshape(D, 1)
        _CACHE["cst"] = (cst_rep, cst_col)
    return _CACHE["cst"]


def kernel(X, d, parameter_t, fm_mean, gamma, beta):
    from concourse.bass_utils import run_bass_kernel_spmd

    X = np.ascontiguousarray(np.asarray(X), dtype=np.float32)
    d = np.ascontiguousarray(np.asarray(d), dtype=np.int32)
    gamma = np.ascontiguousarray(np.asarray(gamma), dtype=np.float32)
    beta = np.ascontiguousarray(np.asarray(beta), dtype=np.float32)

    nc = _get_program()
    cst_rep, cst_col = _constants()
    in_maps = [
        {
            "X": X[c * SHARD:(c + 1) * SHARD],
            "d": d[c * SHARD:(c + 1) * SHARD],
            "gamma": gamma,
            "beta": beta,
            "cst_rep": cst_rep,
            "cst_col": cst_col,
        }
        for c in range(NCORES)
    ]
    res = run_bass_kernel_spmd(nc, in_maps, core_ids=list(range(NCORES)))
    out = np.concatenate([res.results[c]["Y"] for c in range(NCORES)], axis=0)
    return out.astype(np.float32, copy=False)


# revision 31
# speedup vs baseline: 1.2602x; 1.1146x over previous
"""Per-domain batch normalization (BaseDomainBatchNorm) on 8 Trainium2 NeuronCores.

Math (reference):
    cnt[j]   = #{n : d[n] == j}            (clamped to >= 1)
    mean[j]  = sum_{d[n]==j} X[n] / cnt[j]
    var[j]   = sum_{d[n]==j} X[n]^2 / cnt[j] - mean[j]^2
    inv[j]   = rsqrt(var[j] + 1e-5)
    Y[n]     = X[n] * A[d[n]] + B[d[n]],  A = inv*gamma, B = beta - mean*A

Sharding: rows split 8192 per core; per-domain partial stats (sum/sumsq/cnt)
are AllReduce'd across the 8 cores; each core normalizes its own rows.

V2 design (vs the fp32 baseline):
  - X is loaded ONCE as bf16 via SWDGE cast-DMA in 8x 2MB transfers
    (fp32 HBM -> bf16 SBUF); no per-chunk f32->bf16 DVE casts.
  - stats per chunk: psum_s += onehot.T @ x_bf ; psum_q += onehot.T @ xsq
    (xsq alternates ACT Square / DVE mul to split engine load).
  - a tiny warm-up AllReduce issues at t~0 so the ~40us first-collective
    setup cost overlaps the X loads; the real stats AllReduce then runs
    at its marginal latency.
  - phase 2 per super-chunk (2 chunks, [128,2048] psum):
    4 gather matmuls produce [A(c0)|A(c1)|B(c0)|B(c1)] in one psum tile;
    one fp32->bf16 evacuation (ACT for most supers, DVE for the rest to
    balance); FMA on DVE in bf16 at 2x rate; result staged in bf16 and
    cast-stored (bf16 SBUF -> fp32 HBM) in 2MB transfers.
HBM traffic is the roofline minimum: read X once, write Y once.
"""

import numpy as np

N = 65536
C = 512
D = 16
NCORES = 8
SHARD = N // NCORES          # 8192 rows per core
P = 128                      # partitions
CHUNKS = SHARD // P          # 64 chunks of 128 rows
SUPERS = CHUNKS // 2         # 32 super-chunks
GROUPS = CHUNKS // 8         # 8 groups of 8 chunks (2MB DMA granularity)
EPS = 1e-5

_CACHE = {}


def _build_program():
    import concourse.bacc as bacc
    import concourse.tile as tile
    from concourse import mybir

    f32 = mybir.dt.float32
    bf16 = mybir.dt.bfloat16
    i32 = mybir.dt.int32
    Alu = mybir.AluOpType
    Act = mybir.ActivationFunctionType

    nc = bacc.Bacc("TRN2", target_bir_lowering=False, debug=False,
                   num_devices=NCORES)

    X_d = nc.dram_tensor("X", [SHARD, C], f32, kind="ExternalInput")
    d_d = nc.dram_tensor("d", [SHARD], i32, kind="ExternalInput")
    g_d = nc.dram_tensor("gamma", [D, C], f32, kind="ExternalInput")
    b_d = nc.dram_tensor("beta", [D, C], f32, kind="ExternalInput")
    # host-provided compile-time-constant iotas (avoids gpsimd iota ops,
    # whose library load would delay the cast-load DMA queue)
    cr_d = nc.dram_tensor("cst_rep", [P, CHUNKS * D], bf16,
                          kind="ExternalInput")
    cc_d = nc.dram_tensor("cst_col", [D, 1], f32, kind="ExternalInput")
    Y_d = nc.dram_tensor("Y", [SHARD, C], f32, kind="ExternalOutput")

    cc_in = nc.dram_tensor("cc_in", [D, 2 * C + 1], f32)
    cc_out = nc.dram_tensor("cc_out", [D, 2 * C + 1], f32, addr_space="Shared")

    # partition p owns rows [p*64, (p+1)*64)
    Xv = X_d.ap().rearrange("(p n) c -> p n c", p=P)   # [128, 64, 512]
    Yv = Y_d.ap().rearrange("(p n) c -> p n c", p=P)

    DB = 1024  # d-broadcast strip width

    with tile.TileContext(nc) as tc:
        with (
            tc.tile_pool(name="const", bufs=1) as cpool,
            tc.tile_pool(name="x", bufs=GROUPS) as xpool,
            tc.tile_pool(name="sq", bufs=3) as sqpool,
            tc.tile_pool(name="oh", bufs=1) as ohpool,
            tc.tile_pool(name="small", bufs=1) as spool,
            tc.tile_pool(name="scr", bufs=2) as scrpool,
            tc.tile_pool(name="dbc", bufs=2) as dbcpool,
            tc.tile_pool(name="ab", bufs=3) as abpool,
            tc.tile_pool(name="y", bufs=3) as ypool,
        ):
            # ---- X loads: fp32->bf16 cast-DMAs on the gpsimd queue; the
            # last group split in half to shorten the stats tail ----
            d_pn = cpool.tile([P, CHUNKS], i32)
            nc.sync.dma_start(d_pn[:], d_d.ap().rearrange("(p n) -> p n", p=P))
            xs = []
            for g in range(GROUPS):
                xt = xpool.tile([P, 8 * C], bf16)
                xs.append(xt)
            for g in range(GROUPS - 1):
                nc.gpsimd.dma_start(
                    xs[g][:].rearrange("p (n c) -> p n c", c=C),
                    Xv[:, 8 * g:8 * g + 8, :])
            gl = GROUPS - 1
            for h in range(2):
                nc.gpsimd.dma_start(
                    xs[gl][:, h * 4 * C:(h + 1) * 4 * C].rearrange(
                        "p (n c) -> p n c", c=C),
                    Xv[:, 8 * gl + 4 * h:8 * gl + 4 * h + 4, :])

            # ---- constants (scalar queue) ----
            iota_rep = cpool.tile([P, CHUNKS, D], bf16)
            nc.scalar.dma_start(
                iota_rep[:].rearrange("p n d -> p (n d)"), cr_d.ap())
            iota_col32 = cpool.tile([D, 1], f32)
            nc.scalar.dma_start(iota_col32[:], cc_d.ap())
            ones_col = cpool.tile([P, 1], bf16)
            nc.vector.memset(ones_col[:], 1.0)

            # ---- one-hot [128, 64, 16] in chunk layout ----
            d_f = cpool.tile([P, CHUNKS], bf16)
            nc.vector.tensor_copy(d_f[:], d_pn[:])
            onehot = ohpool.tile([P, CHUNKS, D], bf16)
            nc.vector.tensor_tensor(
                onehot[:], iota_rep[:],
                d_f[:].unsqueeze(-1).broadcast_to([P, CHUNKS, D]),
                Alu.is_equal)

            # ---- AB2 table [128, 1024]: rows 0:16 = [A | B], rest zero ----
            AB2 = spool.tile([P, 2 * C], bf16, tag="AB2")
            for pb in range(2 * D, P, 2 * D):
                nc.vector.memset(AB2[pb:pb + 2 * D, :], 0.0)
            nc.scalar.dma_start(AB2[D:2 * D, :], AB2[2 * D:2 * D + D, :])

            # ---- gamma/beta early loads ----
            gam = spool.tile([D, C], f32, tag="gam")
            nc.scalar.dma_start(gam[:], g_d[:])
            bet = spool.tile([D, C], f32, tag="bet")
            nc.scalar.dma_start(bet[:], b_d[:])

            # ---- phase 1: per-core partial stats ----
            stats = spool.tile([D, 2 * C + 1], f32, tag="stats")
            with tc.tile_pool(name="ps1", bufs=1, space="PSUM") as ps1:
                psum_s = ps1.tile([D, C], f32)
                psum_q = ps1.tile([D, C], f32)
                psum_c = ps1.tile([D, 1], f32)
                for i in range(CHUNKS):
                    g, k = divmod(i, 8)
                    xsl = xs[g][:, k * C:(k + 1) * C]
                    xsq = sqpool.tile([P, C], bf16, tag="xsq")
                    if i % 2 == 0:
                        nc.scalar.activation(xsq[:], xsl, Act.Square)
                    else:
                        nc.vector.tensor_mul(xsq[:], xsl, xsl)
                    oh = onehot[:, i, :]
                    st, sp = (i == 0), (i == CHUNKS - 1)
                    nc.tensor.matmul(psum_s[:], oh, xsl,
                                     start=st, stop=sp)
                    nc.tensor.matmul(psum_q[:], oh, xsq[:],
                                     start=st, stop=sp)

                # counts: reduce one-hot over chunks, then one matmul
                rowcnt = spool.tile([P, D], f32, tag="rowcnt")
                nc.vector.tensor_reduce(
                    rowcnt[:], onehot[:].rearrange("p n d -> p d n"),
                    mybir.AxisListType.X, Alu.add)
                rowcnt_bf = spool.tile([P, D], bf16, tag="rowcnt_bf")
                nc.vector.tensor_copy(rowcnt_bf[:], rowcnt[:])
                nc.tensor.matmul(psum_c[:], rowcnt_bf[:], ones_col[:],
                                 start=True, stop=True)

                nc.vector.tensor_copy(stats[:, 0:C], psum_s[:])
                nc.vector.tensor_copy(stats[:, C:2 * C], psum_q[:])
                nc.vector.tensor_copy(stats[:, 2 * C:2 * C + 1], psum_c[:])

                # keep the PE HAM clock-gate warm across the all-reduce stall
                warm = ps1.tile([P, C], f32)
                iflat = iota_rep[:].rearrange("p n d -> p (n d)")
                for _ in range(32):
                    nc.tensor.matmul(warm[:], iflat[:, 0:P],
                                     iflat[:, 0:C],
                                     start=True, stop=True,
                                     skip_group_check=True)

            # ---- transposed one-hot [128, 8192], built during the
            # all-reduce window: rows 0:16 via broadcast strips + is_equal,
            # then partition-doubling SBUF copies to fill 16:128 (their
            # table rows are zero).  The gpsimd queue is free of X loads
            # by now. ----
            onehotT = ohpool.tile([P, SHARD], bf16)
            for h in range(SHARD // DB):
                d_bc = dbcpool.tile([D, DB], i32)
                src = d_d.ap()[h * DB:(h + 1) * DB]
                src = src.rearrange("(a n) -> a n", a=1).partition_broadcast(D)
                nc.gpsimd.dma_start(d_bc[:], src)
                nc.vector.tensor_scalar(onehotT[0:D, h * DB:(h + 1) * DB],
                                        d_bc[:], iota_col32[:], None,
                                        Alu.is_equal)
            for pb in (D, 2 * D, 4 * D):
                nc.gpsimd.dma_start(onehotT[pb:2 * pb, :], onehotT[0:pb, :])

            # ---- all-reduce partial stats across the 8 cores ----
            nc.sync.dma_start(cc_in[:], stats[:])
            nc.gpsimd.collective_compute(
                "AllReduce", Alu.add,
                replica_groups=[list(range(NCORES))],
                ins=[cc_in[:]], outs=[cc_out[:]])
            red = spool.tile([D, 2 * C + 1], f32, tag="red")
            nc.sync.dma_start(red[:], cc_out[:])

            # ---- finalize (reciprocal-free):
            #   rsq = rsqrt(q*cnt - s^2 + eps*cnt^2)
            #   A   = gamma * cnt * rsq
            #   B   = beta - s * gamma * rsq
            # (identical to mean/var/rsqrt form for cnt >= 1)
            cntc = spool.tile([D, 1], f32, tag="cntc")
            nc.vector.tensor_scalar_max(cntc[:], red[:, 2 * C:2 * C + 1], 1.0)
            c2e = spool.tile([D, 1], f32, tag="c2e")
            nc.vector.tensor_tensor(c2e[:], cntc[:], cntc[:], Alu.mult)
            nc.vector.tensor_scalar_mul(c2e[:], c2e[:], EPS)
            t1 = scrpool.tile([D, C], f32, tag="scr")
            nc.vector.tensor_scalar_mul(t1[:], red[:, C:2 * C], cntc[:])
            t2 = spool.tile([D, C], f32, tag="t2")
            nc.vector.scalar_tensor_tensor(t2[:], red[:, 0:C], -1.0,
                                           red[:, 0:C], Alu.mult, Alu.mult)
            nc.vector.tensor_add(t1[:], t1[:], t2[:])   # q*cnt - s^2
            sd = scrpool.tile([D, C], f32, tag="scr")
            nc.scalar.activation(sd[:], t1[:], Act.Sqrt, bias=c2e[:])
            rsq = spool.tile([D, C], f32, tag="rsq")
            nc.vector.reciprocal(rsq[:], sd[:])
            ag = spool.tile([D, C], f32, tag="ag")
            nc.vector.tensor_mul(ag[:], gam[:], rsq[:])       # gamma*rsq
            a_t = spool.tile([D, C], f32, tag="a_t")
            nc.vector.tensor_scalar_mul(a_t[:], ag[:], cntc[:])
            b_t = spool.tile([D, C], f32, tag="b_t")
            nc.vector.scalar_tensor_tensor(b_t[:], red[:, 0:C], -1.0, ag[:],
                                           Alu.mult, Alu.mult)   # -s*gamma*rsq
            nc.vector.tensor_add(b_t[:], bet[:], b_t[:])

            nc.vector.tensor_copy(AB2[0:D, 0:C], a_t[:])
            nc.vector.tensor_copy(AB2[0:D, C:2 * C], b_t[:])

            # ---- phase 2: gather A/B rows (K=128 matmuls), FMA in bf16,
            # cast-store.  A/B gathers land in SEPARATE psum tiles and are
            # evacuated separately so the pipeline releases resources
            # early.  Two schemes balance ACT vs DVE:
            #   A: ACT evacs A and B (2x FD=1024); DVE mul+add bf16 (2x)
            #   B: ACT evacs A only; DVE mul bf16 + add from PSUM (1x)
            ohTv = onehotT[:].rearrange("k (p i) -> k i p", i=CHUNKS)
            with tc.tile_pool(name="ps2", bufs=2, space="PSUM") as ps2:
                for g in range(GROUPS):
                    yb = ypool.tile([P, 8 * C], bf16)
                    for s in range(4 * g, 4 * g + 4):
                        q = s % 4
                        pA = ps2.tile([P, 2 * C], f32, tag="pA")
                        pB = ps2.tile([P, 2 * C], f32, tag="pB")
                        lt0 = ohTv[:, 2 * s, :]
                        lt1 = ohTv[:, 2 * s + 1, :]
                        nc.tensor.matmul(pA[:, 0:C], lt0, AB2[:, 0:C],
                                         start=True, stop=True)
                        nc.tensor.matmul(pA[:, C:2 * C], lt1, AB2[:, 0:C],
                                         start=True, stop=True)
                        nc.tensor.matmul(pB[:, 0:C], lt0, AB2[:, C:2 * C],
                                         start=True, stop=True)
                        nc.tensor.matmul(pB[:, C:2 * C], lt1, AB2[:, C:2 * C],
                                         start=True, stop=True)
                        ysl = yb[:, q * 2 * C:(q + 1) * 2 * C]
                        xsl = xs[g][:, q * 2 * C:(q + 1) * 2 * C]
                        a_sb = abpool.tile([P, 2 * C], bf16, tag="a_sb")
                        nc.scalar.activation(a_sb[:], pA[:], Act.Copy)
                        nc.vector.tensor_mul(ysl, xsl, a_sb[:])
                        if (s % 16) % 2 == 1 or (s % 16) == 14:
                            nc.vector.tensor_add(ysl, ysl, pB[:])
                        else:
                            b_sb = abpool.tile([P, 2 * C], bf16, tag="b_sb")
                            nc.scalar.activation(b_sb[:], pB[:], Act.Copy)
                            nc.vector.tensor_add(ysl, ysl, b_sb[:])
                    nc.gpsimd.dma_start(
                        Yv[:, 8 * g:8 * g + 8, :],
                        yb[:].rearrange("p (n c) -> p n c", c=C))

    nc.compile()
    return nc


def _get_program():
    if "nc" not in _CACHE:
        _CACHE["nc"] = _build_program()
    return _CACHE["nc"]


def _constants():
    if "cst" not in _CACHE:
        import ml_dtypes
        rep = np.tile(np.arange(D, dtype=np.float32), (P, CHUNKS))
        cst_rep = rep.astype(ml_dtypes.bfloat16)       # [P, CHUNKS*D]
        cst_col = np.arange(D, dtype=np.float32).reshape(D, 1)
        _CACHE["cst"] = (cst_rep, cst_col)
    return _CACHE["cst"]


def kernel(X, d, parameter_t, fm_mean, gamma, beta):
    from concourse.bass_utils import run_bass_kernel_spmd

    X = np.ascontiguousarray(np.asarray(X), dtype=np.float32)
    d = np.ascontiguousarray(np.asarray(d), dtype=np.int32)
    gamma = np.ascontiguousarray(np.asarray(gamma), dtype=np.float32)
    beta = np.ascontiguousarray(np.asarray(beta), dtype=np.float32)

    nc = _get_program()
    cst_rep, cst_col = _constants()
    in_maps = [
        {
            "X": X[c * SHARD:(c + 1) * SHARD],
            "d": d[c * SHARD:(c + 1) * SHARD],
            "gamma": gamma,
            "beta": beta,
            "cst_rep": cst_rep,
            "cst_col": cst_col,
        }
        for c in range(NCORES)
    ]
    res = run_bass_kernel_spmd(nc, in_maps, core_ids=list(range(NCORES)))
    out = np.concatenate([res.results[c]["Y"] for c in range(NCORES)], axis=0)
    return out.astype(np.float32, copy=False)


# revision 36
# speedup vs baseline: 1.3224x; 1.0494x over previous
"""Per-domain batch normalization (BaseDomainBatchNorm) on 8 Trainium2 NeuronCores.

Math (reference):
    cnt[j]   = #{n : d[n] == j}            (clamped to >= 1)
    mean[j]  = sum_{d[n]==j} X[n] / cnt[j]
    var[j]   = sum_{d[n]==j} X[n]^2 / cnt[j] - mean[j]^2
    inv[j]   = rsqrt(var[j] + 1e-5)
    Y[n]     = X[n] * A[d[n]] + B[d[n]],  A = inv*gamma, B = beta - mean*A

Sharding: rows split 8192 per core; per-domain partial stats (sum/sumsq/cnt)
are AllReduce'd across the 8 cores; each core normalizes its own rows.

V2 design (vs the fp32 baseline):
  - X is loaded ONCE as bf16 via SWDGE cast-DMA in 8x 2MB transfers
    (fp32 HBM -> bf16 SBUF); no per-chunk f32->bf16 DVE casts.
  - stats per chunk: psum_s += onehot.T @ x_bf ; psum_q += onehot.T @ xsq
    (xsq alternates ACT Square / DVE mul to split engine load).
  - a tiny warm-up AllReduce issues at t~0 so the ~40us first-collective
    setup cost overlaps the X loads; the real stats AllReduce then runs
    at its marginal latency.
  - phase 2 per super-chunk (2 chunks, [128,2048] psum):
    4 gather matmuls produce [A(c0)|A(c1)|B(c0)|B(c1)] in one psum tile;
    one fp32->bf16 evacuation (ACT for most supers, DVE for the rest to
    balance); FMA on DVE in bf16 at 2x rate; result staged in bf16 and
    cast-stored (bf16 SBUF -> fp32 HBM) in 2MB transfers.
HBM traffic is the roofline minimum: read X once, write Y once.
"""

import numpy as np

N = 65536
C = 512
D = 16
NCORES = 8
SHARD = N // NCORES          # 8192 rows per core
P = 128                      # partitions
CHUNKS = SHARD // P          # 64 chunks of 128 rows
SUPERS = CHUNKS // 2         # 32 super-chunks
GROUPS = CHUNKS // 8         # 8 groups of 8 chunks (2MB DMA granularity)
EPS = 1e-5

_CACHE = {}


def _build_program():
    import concourse.bacc as bacc
    import concourse.tile as tile
    from concourse import mybir

    f32 = mybir.dt.float32
    bf16 = mybir.dt.bfloat16
    i32 = mybir.dt.int32
    Alu = mybir.AluOpType
    Act = mybir.ActivationFunctionType

    nc = bacc.Bacc("TRN2", target_bir_lowering=False, debug=False,
                   num_devices=NCORES)

    X_d = nc.dram_tensor("X", [SHARD, C], f32, kind="ExternalInput")
    d_d = nc.dram_tensor("d", [SHARD], i32, kind="ExternalInput")
    g_d = nc.dram_tensor("gamma", [D, C], f32, kind="ExternalInput")
    b_d = nc.dram_tensor("beta", [D, C], f32, kind="ExternalInput")
    # host-provided compile-time-constant iotas (avoids gpsimd iota ops,
    # whose library load would delay the cast-load DMA queue)
    cr_d = nc.dram_tensor("cst_rep", [P, CHUNKS * D], bf16,
                          kind="ExternalInput")
    cc_d = nc.dram_tensor("cst_col", [D, 1], f32, kind="ExternalInput")
    Y_d = nc.dram_tensor("Y", [SHARD, C], f32, kind="ExternalOutput")

    ccw_in = nc.dram_tensor("ccw_in", [D, 1], f32)
    ccw_out = nc.dram_tensor("ccw_out", [D, 1], f32, addr_space="Shared")
    cc_in = nc.dram_tensor("cc_in", [D, 2 * C + 1], bf16)
    cc_out = nc.dram_tensor("cc_out", [D, 2 * C + 1], bf16,
                            addr_space="Shared")

    # partition p owns rows [p*64, (p+1)*64)
    Xv = X_d.ap().rearrange("(p n) c -> p n c", p=P)   # [128, 64, 512]
    Yv = Y_d.ap().rearrange("(p n) c -> p n c", p=P)

    DB = 1024  # d-broadcast strip width

    with tile.TileContext(nc) as tc:
        with (
            tc.tile_pool(name="const", bufs=1) as cpool,
            tc.tile_pool(name="x", bufs=GROUPS) as xpool,
            tc.tile_pool(name="sq", bufs=3) as sqpool,
            tc.tile_pool(name="oh", bufs=1) as ohpool,
            tc.tile_pool(name="small", bufs=1) as spool,
            tc.tile_pool(name="scr", bufs=2) as scrpool,
            tc.tile_pool(name="dbc", bufs=2) as dbcpool,
            tc.tile_pool(name="ab", bufs=3) as abpool,
            tc.tile_pool(name="y", bufs=3) as ypool,
        ):
            # ---- warm-up collective: pays the barrier + first-collective
            # setup while the X loads stream in ----
            warm_t = spool.tile([D, 1], f32, tag="warm")
            nc.vector.memset(warm_t[:], 1.0)
            nc.scalar.dma_start(ccw_in[:], warm_t[:])
            nc.gpsimd.collective_compute(
                "AllReduce", Alu.add,
                replica_groups=[list(range(NCORES))],
                ins=[ccw_in[:]], outs=[ccw_out[:]])

            # ---- X loads: fp32->bf16 cast-DMAs on the gpsimd queue; the
            # last group split in half to shorten the stats tail ----
            d_pn = cpool.tile([P, CHUNKS], i32)
            nc.sync.dma_start(d_pn[:], d_d.ap().rearrange("(p n) -> p n", p=P))
            xs = []
            for g in range(GROUPS):
                xt = xpool.tile([P, 8 * C], bf16)
                xs.append(xt)
            for g in range(GROUPS - 1):
                nc.gpsimd.dma_start(
                    xs[g][:].rearrange("p (n c) -> p n c", c=C),
                    Xv[:, 8 * g:8 * g + 8, :])
            gl = GROUPS - 1
            for h in range(2):
                nc.gpsimd.dma_start(
                    xs[gl][:, h * 4 * C:(h + 1) * 4 * C].rearrange(
                        "p (n c) -> p n c", c=C),
                    Xv[:, 8 * gl + 4 * h:8 * gl + 4 * h + 4, :])

            # ---- constants (scalar queue) ----
            iota_rep = cpool.tile([P, CHUNKS, D], bf16)
            nc.scalar.dma_start(
                iota_rep[:].rearrange("p n d -> p (n d)"), cr_d.ap())
            iota_col32 = cpool.tile([D, 1], f32)
            nc.scalar.dma_start(iota_col32[:], cc_d.ap())
            ones_col = cpool.tile([P, 1], bf16)
            nc.vector.memset(ones_col[:], 1.0)

            # ---- one-hot [128, 64, 16] in chunk layout ----
            d_f = cpool.tile([P, CHUNKS], bf16)
            nc.vector.tensor_copy(d_f[:], d_pn[:])
            onehot = ohpool.tile([P, CHUNKS, D], bf16)
            nc.vector.tensor_tensor(
                onehot[:], iota_rep[:],
                d_f[:].unsqueeze(-1).broadcast_to([P, CHUNKS, D]),
                Alu.is_equal)

            # ---- AB2 table [128, 1024]: rows 0:16 = [A | B], rest zero ----
            AB2 = spool.tile([P, 2 * C], bf16, tag="AB2")
            for pb in range(2 * D, P, 2 * D):
                nc.vector.memset(AB2[pb:pb + 2 * D, :], 0.0)
            nc.scalar.dma_start(AB2[D:2 * D, :], AB2[2 * D:2 * D + D, :])

            # ---- gamma/beta early loads ----
            gam = spool.tile([D, C], f32, tag="gam")
            nc.scalar.dma_start(gam[:], g_d[:])
            bet = spool.tile([D, C], f32, tag="bet")
            nc.scalar.dma_start(bet[:], b_d[:])

            # ---- phase 1: per-core partial stats (AR payload in bf16) ----
            stats = spool.tile([D, 2 * C + 1], bf16, tag="stats")
            with tc.tile_pool(name="ps1", bufs=1, space="PSUM") as ps1:
                psum_s = ps1.tile([D, C], f32)
                psum_q = ps1.tile([D, C], f32)
                psum_c = ps1.tile([D, 1], f32)
                for i in range(CHUNKS):
                    g, k = divmod(i, 8)
                    xsl = xs[g][:, k * C:(k + 1) * C]
                    xsq = sqpool.tile([P, C], bf16, tag="xsq")
                    if i % 2 == 0:
                        nc.scalar.activation(xsq[:], xsl, Act.Square)
                    else:
                        nc.vector.tensor_mul(xsq[:], xsl, xsl)
                    oh = onehot[:, i, :]
                    st, sp = (i == 0), (i == CHUNKS - 1)
                    nc.tensor.matmul(psum_s[:], oh, xsl,
                                     start=st, stop=sp)
                    nc.tensor.matmul(psum_q[:], oh, xsq[:],
                                     start=st, stop=sp)

                # counts: reduce one-hot over chunks, then one matmul
                rowcnt = spool.tile([P, D], f32, tag="rowcnt")
                nc.vector.tensor_reduce(
                    rowcnt[:], onehot[:].rearrange("p n d -> p d n"),
                    mybir.AxisListType.X, Alu.add)
                rowcnt_bf = spool.tile([P, D], bf16, tag="rowcnt_bf")
                nc.vector.tensor_copy(rowcnt_bf[:], rowcnt[:])
                nc.tensor.matmul(psum_c[:], rowcnt_bf[:], ones_col[:],
                                 start=True, stop=True)

                nc.vector.tensor_copy(stats[:, 0:C], psum_s[:])
                nc.vector.tensor_copy(stats[:, C:2 * C], psum_q[:])
                nc.vector.tensor_copy(stats[:, 2 * C:2 * C + 1], psum_c[:])

                # keep the PE HAM clock-gate warm across the all-reduce stall
                warm = ps1.tile([P, C], f32)
                iflat = iota_rep[:].rearrange("p n d -> p (n d)")
                for _ in range(32):
                    nc.tensor.matmul(warm[:], iflat[:, 0:P],
                                     iflat[:, 0:C],
                                     start=True, stop=True,
                                     skip_group_check=True)

            # ---- transposed one-hot [128, 8192], built during the
            # all-reduce window: rows 0:16 via broadcast strips + is_equal,
            # then partition-doubling SBUF copies to fill 16:128 (their
            # table rows are zero).  The gpsimd queue is free of X loads
            # by now. ----
            onehotT = ohpool.tile([P, SHARD], bf16)
            for h in range(SHARD // DB):
                d_bc = dbcpool.tile([D, DB], i32)
                src = d_d.ap()[h * DB:(h + 1) * DB]
                src = src.rearrange("(a n) -> a n", a=1).partition_broadcast(D)
                nc.gpsimd.dma_start(d_bc[:], src)
                nc.vector.tensor_scalar(onehotT[0:D, h * DB:(h + 1) * DB],
                                        d_bc[:], iota_col32[:], None,
                                        Alu.is_equal)
            for pb in (D, 2 * D, 4 * D):
                nc.gpsimd.dma_start(onehotT[pb:2 * pb, :], onehotT[0:pb, :])

            # ---- all-reduce partial stats across the 8 cores ----
            nc.sync.dma_start(cc_in[:], stats[:])
            nc.gpsimd.collective_compute(
                "AllReduce", Alu.add,
                replica_groups=[list(range(NCORES))],
                ins=[cc_in[:]], outs=[cc_out[:]])
            red = spool.tile([D, 2 * C + 1], bf16, tag="red")
            nc.sync.dma_start(red[:], cc_out[:])

            # ---- finalize (reciprocal-free):
            #   rsq = rsqrt(q*cnt - s^2 + eps*cnt^2)
            #   A   = gamma * cnt * rsq
            #   B   = beta - s * gamma * rsq
            # (identical to mean/var/rsqrt form for cnt >= 1)
            cntc = spool.tile([D, 1], f32, tag="cntc")
            nc.vector.tensor_scalar_max(cntc[:], red[:, 2 * C:2 * C + 1], 1.0)
            c2e = spool.tile([D, 1], f32, tag="c2e")
            nc.vector.tensor_tensor(c2e[:], cntc[:], cntc[:], Alu.mult)
            nc.vector.tensor_scalar_mul(c2e[:], c2e[:], EPS)
            t1 = scrpool.tile([D, C], f32, tag="scr")
            nc.vector.tensor_scalar_mul(t1[:], red[:, C:2 * C], cntc[:])
            t2 = spool.tile([D, C], f32, tag="t2")
            nc.vector.scalar_tensor_tensor(t2[:], red[:, 0:C], -1.0,
                                           red[:, 0:C], Alu.mult, Alu.mult)
            nc.vector.tensor_add(t1[:], t1[:], t2[:])   # q*cnt - s^2
            sd = scrpool.tile([D, C], f32, tag="scr")
            nc.scalar.activation(sd[:], t1[:], Act.Sqrt, bias=c2e[:])
            rsq = spool.tile([D, C], f32, tag="rsq")
            nc.vector.reciprocal(rsq[:], sd[:])
            ag = spool.tile([D, C], f32, tag="ag")
            nc.vector.tensor_mul(ag[:], gam[:], rsq[:])       # gamma*rsq
            a_t = spool.tile([D, C], f32, tag="a_t")
            nc.vector.tensor_scalar_mul(a_t[:], ag[:], cntc[:])
            b_t = spool.tile([D, C], f32, tag="b_t")
            nc.vector.scalar_tensor_tensor(b_t[:], red[:, 0:C], -1.0, ag[:],
                                           Alu.mult, Alu.mult)   # -s*gamma*rsq
            nc.vector.tensor_add(b_t[:], bet[:], b_t[:])

            nc.vector.tensor_copy(AB2[0:D, 0:C], a_t[:])
            nc.vector.tensor_copy(AB2[0:D, C:2 * C], b_t[:])

            # ---- phase 2: gather A/B rows (K=128 matmuls), FMA in bf16,
            # cast-store.  A/B gathers land in SEPARATE psum tiles and are
            # evacuated separately so the pipeline releases resources
            # early.  Two schemes balance ACT vs DVE:
            #   A: ACT evacs A and B (2x FD=1024); DVE mul+add bf16 (2x)
            #   B: ACT evacs A only; DVE mul bf16 + add from PSUM (1x)
            ohTv = onehotT[:].rearrange("k (p i) -> k i p", i=CHUNKS)
            with tc.tile_pool(name="ps2", bufs=2, space="PSUM") as ps2:
                for g in range(GROUPS):
                    yb = ypool.tile([P, 8 * C], bf16)
                    for s in range(4 * g, 4 * g + 4):
                        q = s % 4
                        pA = ps2.tile([P, 2 * C], f32, tag="pA")
                        pB = ps2.tile([P, 2 * C], f32, tag="pB")
                        lt0 = ohTv[:, 2 * s, :]
                        lt1 = ohTv[:, 2 * s + 1, :]
                        nc.tensor.matmul(pA[:, 0:C], lt0, AB2[:, 0:C],
                                         start=True, stop=True)
                        nc.tensor.matmul(pA[:, C:2 * C], lt1, AB2[:, 0:C],
                                         start=True, stop=True)
                        nc.tensor.matmul(pB[:, 0:C], lt0, AB2[:, C:2 * C],
                                         start=True, stop=True)
                        nc.tensor.matmul(pB[:, C:2 * C], lt1, AB2[:, C:2 * C],
                                         start=True, stop=True)
                        ysl = yb[:, q * 2 * C:(q + 1) * 2 * C]
                        xsl = xs[g][:, q * 2 * C:(q + 1) * 2 * C]
                        a_sb = abpool.tile([P, 2 * C], bf16, tag="a_sb")
                        nc.scalar.activation(a_sb[:], pA[:], Act.Copy)
                        nc.vector.tensor_mul(ysl, xsl, a_sb[:])
                        if (s % 16) % 2 == 1 or (s % 16) == 14:
                            nc.vector.tensor_add(ysl, ysl, pB[:])
                        else:
                            b_sb = abpool.tile([P, 2 * C], bf16, tag="b_sb")
                            nc.scalar.activation(b_sb[:], pB[:], Act.Copy)
                            nc.vector.tensor_add(ysl, ysl, b_sb[:])
                    if g < GROUPS - 1:
                        nc.gpsimd.dma_start(
                            Yv[:, 8 * g:8 * g + 8, :],
                            yb[:].rearrange("p (n c) -> p n c", c=C))
                    else:
                        # split the last store so its first half overlaps
                        # the final FMAs
                        for h in range(2):
                            nc.gpsimd.dma_start(
                                Yv[:, 8 * g + 4 * h:8 * g + 4 * h + 4, :],
                                yb[:, h * 4 * C:(h + 1) * 4 * C].rearrange(
                                    "p (n c) -> p n c", c=C))

    nc.compile()
    return nc


def _get_program():
    if "nc" not in _CACHE:
        _CACHE["nc"] = _build_program()
    return _CACHE["nc"]


def _constants():
    if "cst" not in _CACHE:
        import ml_dtypes
        rep = np.tile(np.arange(D, dtype=np.float32), (P, CHUNKS))
        cst_rep = rep.astype(ml_dtypes.bfloat16)       # [P, CHUNKS*D]
        cst_col = np.arange(D, dtype=np.float32).reshape(D, 1)
        _CACHE["cst"] = (cst_rep, cst_col)
    return _CACHE["cst"]


def kernel(X, d, parameter_t, fm_mean, gamma, beta):
    from concourse.bass_utils import run_bass_kernel_spmd

    X = np.ascontiguousarray(np.asarray(X), dtype=np.float32)
    d = np.ascontiguousarray(np.asarray(d), dtype=np.int32)
    gamma = np.ascontiguousarray(np.asarray(gamma), dtype=np.float32)
    beta = np.ascontiguousarray(np.asarray(beta), dtype=np.float32)

    nc = _get_program()
    cst_rep, cst_col = _constants()
    in_maps = [
        {
            "X": X[c * SHARD:(c + 1) * SHARD],
            "d": d[c * SHARD:(c + 1) * SHARD],
            "gamma": gamma,
            "beta": beta,
            "cst_rep": cst_rep,
            "cst_col": cst_col,
        }
        for c in range(NCORES)
    ]
    res = run_bass_kernel_spmd(nc, in_maps, core_ids=list(range(NCORES)))
    out = np.concatenate([res.results[c]["Y"] for c in range(NCORES)], axis=0)
    return out.astype(np.float32, copy=False)


# revision 37
# speedup vs baseline: 1.4369x; 1.0865x over previous
"""Per-domain batch normalization (BaseDomainBatchNorm) on 8 Trainium2 NeuronCores.

Math (reference):
    cnt[j]   = #{n : d[n] == j}            (clamped to >= 1)
    mean[j]  = sum_{d[n]==j} X[n] / cnt[j]
    var[j]   = sum_{d[n]==j} X[n]^2 / cnt[j] - mean[j]^2
    inv[j]   = rsqrt(var[j] + 1e-5)
    Y[n]     = X[n] * A[d[n]] + B[d[n]],  A = inv*gamma, B = beta - mean*A

Sharding: rows split 8192 per core; per-domain partial stats (sum/sumsq/cnt)
are AllReduce'd across the 8 cores; each core normalizes its own rows.

V2 design (vs the fp32 baseline):
  - X is loaded ONCE as bf16 via SWDGE cast-DMA in 8x 2MB transfers
    (fp32 HBM -> bf16 SBUF); no per-chunk f32->bf16 DVE casts.
  - stats per chunk: psum_s += onehot.T @ x_bf ; psum_q += onehot.T @ xsq
    (xsq alternates ACT Square / DVE mul to split engine load).
  - a tiny warm-up AllReduce issues at t~0 so the ~40us first-collective
    setup cost overlaps the X loads; the real stats AllReduce then runs
    at its marginal latency.
  - phase 2 per super-chunk (2 chunks, [128,2048] psum):
    4 gather matmuls produce [A(c0)|A(c1)|B(c0)|B(c1)] in one psum tile;
    one fp32->bf16 evacuation (ACT for most supers, DVE for the rest to
    balance); FMA on DVE in bf16 at 2x rate; result staged in bf16 and
    cast-stored (bf16 SBUF -> fp32 HBM) in 2MB transfers.
HBM traffic is the roofline minimum: read X once, write Y once.
"""

import numpy as np

N = 65536
C = 512
D = 16
NCORES = 8
SHARD = N // NCORES          # 8192 rows per core
P = 128                      # partitions
CHUNKS = SHARD // P          # 64 chunks of 128 rows
SUPERS = CHUNKS // 2         # 32 super-chunks
GROUPS = CHUNKS // 8         # 8 groups of 8 chunks (2MB DMA granularity)
EPS = 1e-5

_CACHE = {}


def _build_program():
    import concourse.bacc as bacc
    import concourse.tile as tile
    from concourse import mybir

    f32 = mybir.dt.float32
    bf16 = mybir.dt.bfloat16
    i32 = mybir.dt.int32
    Alu = mybir.AluOpType
    Act = mybir.ActivationFunctionType

    nc = bacc.Bacc("TRN2", target_bir_lowering=False, debug=False,
                   num_devices=NCORES)

    X_d = nc.dram_tensor("X", [SHARD, C], f32, kind="ExternalInput")
    d_d = nc.dram_tensor("d", [SHARD], i32, kind="ExternalInput")
    g_d = nc.dram_tensor("gamma", [D, C], f32, kind="ExternalInput")
    b_d = nc.dram_tensor("beta", [D, C], f32, kind="ExternalInput")
    # host-provided compile-time-constant iotas (avoids gpsimd iota ops,
    # whose library load would delay the cast-load DMA queue)
    cr_d = nc.dram_tensor("cst_rep", [P, CHUNKS * D], bf16,
                          kind="ExternalInput")
    cc_d = nc.dram_tensor("cst_col", [D, 1], f32, kind="ExternalInput")
    Y_d = nc.dram_tensor("Y", [SHARD, C], f32, kind="ExternalOutput")

    ccw_in = nc.dram_tensor("ccw_in", [D, 1], f32)
    ccw_out = nc.dram_tensor("ccw_out", [D, 1], f32, addr_space="Shared")
    cc_in = nc.dram_tensor("cc_in", [D, 2 * C + 1], bf16)
    cc_out = nc.dram_tensor("cc_out", [D, 2 * C + 1], bf16,
                            addr_space="Shared")

    # partition p owns rows [p*64, (p+1)*64)
    Xv = X_d.ap().rearrange("(p n) c -> p n c", p=P)   # [128, 64, 512]
    Yv = Y_d.ap().rearrange("(p n) c -> p n c", p=P)

    DB = 1024  # d-broadcast strip width

    with tile.TileContext(nc) as tc:
        with (
            tc.tile_pool(name="const", bufs=1) as cpool,
            tc.tile_pool(name="x", bufs=GROUPS) as xpool,
            tc.tile_pool(name="sq", bufs=3) as sqpool,
            tc.tile_pool(name="oh", bufs=1) as ohpool,
            tc.tile_pool(name="small", bufs=1) as spool,
            tc.tile_pool(name="scr", bufs=2) as scrpool,
            tc.tile_pool(name="dbc", bufs=2) as dbcpool,
            tc.tile_pool(name="ab", bufs=3) as abpool,
            tc.tile_pool(name="y", bufs=3) as ypool,
        ):
            # ---- warm-up collective: pays the barrier + first-collective
            # setup while the X loads stream in.  high_priority pins the
            # trigger to the head of the gpsimd stream (its output is
            # otherwise unconsumed and would be scheduler-sunk). ----
            with tc.high_priority():
                warm_t = spool.tile([D, 1], f32, tag="warm")
                nc.vector.memset(warm_t[:], 1.0)
                nc.scalar.dma_start(ccw_in[:], warm_t[:])
                nc.gpsimd.collective_compute(
                    "AllReduce", Alu.add,
                    replica_groups=[list(range(NCORES))],
                    ins=[ccw_in[:]], outs=[ccw_out[:]])

            # ---- X loads: fp32->bf16 cast-DMAs on the gpsimd queue; the
            # last group split in half to shorten the stats tail ----
            d_pn = cpool.tile([P, CHUNKS], i32)
            nc.sync.dma_start(d_pn[:], d_d.ap().rearrange("(p n) -> p n", p=P))
            xs = []
            for g in range(GROUPS):
                xt = xpool.tile([P, 8 * C], bf16)
                xs.append(xt)
            for g in range(GROUPS - 1):
                nc.gpsimd.dma_start(
                    xs[g][:].rearrange("p (n c) -> p n c", c=C),
                    Xv[:, 8 * g:8 * g + 8, :])
            gl = GROUPS - 1
            for h in range(2):
                nc.gpsimd.dma_start(
                    xs[gl][:, h * 4 * C:(h + 1) * 4 * C].rearrange(
                        "p (n c) -> p n c", c=C),
                    Xv[:, 8 * gl + 4 * h:8 * gl + 4 * h + 4, :])

            # ---- constants (scalar queue) ----
            iota_rep = cpool.tile([P, CHUNKS, D], bf16)
            nc.scalar.dma_start(
                iota_rep[:].rearrange("p n d -> p (n d)"), cr_d.ap())
            iota_col32 = cpool.tile([D, 1], f32)
            nc.scalar.dma_start(iota_col32[:], cc_d.ap())
            ones_col = cpool.tile([P, 1], bf16)
            nc.vector.memset(ones_col[:], 1.0)

            # ---- one-hot [128, 64, 16] in chunk layout ----
            d_f = cpool.tile([P, CHUNKS], bf16)
            nc.vector.tensor_copy(d_f[:], d_pn[:])
            onehot = ohpool.tile([P, CHUNKS, D], bf16)
            nc.vector.tensor_tensor(
                onehot[:], iota_rep[:],
                d_f[:].unsqueeze(-1).broadcast_to([P, CHUNKS, D]),
                Alu.is_equal)

            # ---- AB2 table [128, 1024]: rows 0:16 = [A | B], rest zero ----
            AB2 = spool.tile([P, 2 * C], bf16, tag="AB2")
            for pb in range(2 * D, P, 2 * D):
                nc.vector.memset(AB2[pb:pb + 2 * D, :], 0.0)
            nc.scalar.dma_start(AB2[D:2 * D, :], AB2[2 * D:2 * D + D, :])

            # ---- gamma/beta early loads ----
            gam = spool.tile([D, C], f32, tag="gam")
            nc.scalar.dma_start(gam[:], g_d[:])
            bet = spool.tile([D, C], f32, tag="bet")
            nc.scalar.dma_start(bet[:], b_d[:])

            # ---- phase 1: per-core partial stats (AR payload in bf16) ----
            stats = spool.tile([D, 2 * C + 1], bf16, tag="stats")
            with tc.tile_pool(name="ps1", bufs=1, space="PSUM") as ps1:
                psum_s = ps1.tile([D, C], f32)
                psum_q = ps1.tile([D, C], f32)
                psum_c = ps1.tile([D, 1], f32)
                for i in range(CHUNKS):
                    g, k = divmod(i, 8)
                    xsl = xs[g][:, k * C:(k + 1) * C]
                    xsq = sqpool.tile([P, C], bf16, tag="xsq")
                    if i % 2 == 0:
                        nc.scalar.activation(xsq[:], xsl, Act.Square)
                    else:
                        nc.vector.tensor_mul(xsq[:], xsl, xsl)
                    oh = onehot[:, i, :]
                    st, sp = (i == 0), (i == CHUNKS - 1)
                    nc.tensor.matmul(psum_s[:], oh, xsl,
                                     start=st, stop=sp)
                    nc.tensor.matmul(psum_q[:], oh, xsq[:],
                                     start=st, stop=sp)

                # counts: reduce one-hot over chunks, then one matmul
                rowcnt = spool.tile([P, D], f32, tag="rowcnt")
                nc.vector.tensor_reduce(
                    rowcnt[:], onehot[:].rearrange("p n d -> p d n"),
                    mybir.AxisListType.X, Alu.add)
                rowcnt_bf = spool.tile([P, D], bf16, tag="rowcnt_bf")
                nc.vector.tensor_copy(rowcnt_bf[:], rowcnt[:])
                nc.tensor.matmul(psum_c[:], rowcnt_bf[:], ones_col[:],
                                 start=True, stop=True)

                nc.vector.tensor_copy(stats[:, 0:C], psum_s[:])
                nc.vector.tensor_copy(stats[:, C:2 * C], psum_q[:])
                nc.vector.tensor_copy(stats[:, 2 * C:2 * C + 1], psum_c[:])

                # keep the PE HAM clock-gate warm across the all-reduce stall
                warm = ps1.tile([P, C], f32)
                iflat = iota_rep[:].rearrange("p n d -> p (n d)")
                for _ in range(32):
                    nc.tensor.matmul(warm[:], iflat[:, 0:P],
                                     iflat[:, 0:C],
                                     start=True, stop=True,
                                     skip_group_check=True)

            # ---- transposed one-hot [128, 8192], built during the
            # all-reduce window: rows 0:16 via broadcast strips + is_equal,
            # then partition-doubling SBUF copies to fill 16:128 (their
            # table rows are zero).  The gpsimd queue is free of X loads
            # by now. ----
            onehotT = ohpool.tile([P, SHARD], bf16)
            for h in range(SHARD // DB):
                d_bc = dbcpool.tile([D, DB], i32)
                src = d_d.ap()[h * DB:(h + 1) * DB]
                src = src.rearrange("(a n) -> a n", a=1).partition_broadcast(D)
                nc.gpsimd.dma_start(d_bc[:], src)
                nc.vector.tensor_scalar(onehotT[0:D, h * DB:(h + 1) * DB],
                                        d_bc[:], iota_col32[:], None,
                                        Alu.is_equal)
            for pb in (D, 2 * D, 4 * D):
                nc.gpsimd.dma_start(onehotT[pb:2 * pb, :], onehotT[0:pb, :])

            # ---- all-reduce partial stats across the 8 cores ----
            nc.sync.dma_start(cc_in[:], stats[:])
            nc.gpsimd.collective_compute(
                "AllReduce", Alu.add,
                replica_groups=[list(range(NCORES))],
                ins=[cc_in[:]], outs=[cc_out[:]])
            red = spool.tile([D, 2 * C + 1], bf16, tag="red")
            nc.sync.dma_start(red[:], cc_out[:])

            # ---- finalize (reciprocal-free):
            #   rsq = rsqrt(q*cnt - s^2 + eps*cnt^2)
            #   A   = gamma * cnt * rsq
            #   B   = beta - s * gamma * rsq
            # (identical to mean/var/rsqrt form for cnt >= 1)
            cntc = spool.tile([D, 1], f32, tag="cntc")
            nc.vector.tensor_scalar_max(cntc[:], red[:, 2 * C:2 * C + 1], 1.0)
            c2e = spool.tile([D, 1], f32, tag="c2e")
            nc.vector.tensor_tensor(c2e[:], cntc[:], cntc[:], Alu.mult)
            nc.vector.tensor_scalar_mul(c2e[:], c2e[:], EPS)
            t1 = scrpool.tile([D, C], f32, tag="scr")
            nc.vector.tensor_scalar_mul(t1[:], red[:, C:2 * C], cntc[:])
            t2 = spool.tile([D, C], f32, tag="t2")
            nc.vector.scalar_tensor_tensor(t2[:], red[:, 0:C], -1.0,
                                           red[:, 0:C], Alu.mult, Alu.mult)
            nc.vector.tensor_add(t1[:], t1[:], t2[:])   # q*cnt - s^2
            sd = scrpool.tile([D, C], f32, tag="scr")
            nc.scalar.activation(sd[:], t1[:], Act.Sqrt, bias=c2e[:])
            rsq = spool.tile([D, C], f32, tag="rsq")
            nc.vector.reciprocal(rsq[:], sd[:])
            ag = spool.tile([D, C], f32, tag="ag")
            nc.vector.tensor_mul(ag[:], gam[:], rsq[:])       # gamma*rsq
            a_t = spool.tile([D, C], f32, tag="a_t")
            nc.vector.tensor_scalar_mul(a_t[:], ag[:], cntc[:])
            b_t = spool.tile([D, C], f32, tag="b_t")
            nc.vector.scalar_tensor_tensor(b_t[:], red[:, 0:C], -1.0, ag[:],
                                           Alu.mult, Alu.mult)   # -s*gamma*rsq
            nc.vector.tensor_add(b_t[:], bet[:], b_t[:])

            nc.vector.tensor_copy(AB2[0:D, 0:C], a_t[:])
            nc.vector.tensor_copy(AB2[0:D, C:2 * C], b_t[:])

            # ---- phase 2: gather A/B rows (K=128 matmuls), FMA in bf16,
            # cast-store.  A/B gathers land in SEPARATE psum tiles and are
            # evacuated separately so the pipeline releases resources
            # early.  Two schemes balance ACT vs DVE:
            #   A: ACT evacs A and B (2x FD=1024); DVE mul+add bf16 (2x)
            #   B: ACT evacs A only; DVE mul bf16 + add from PSUM (1x)
            ohTv = onehotT[:].rearrange("k (p i) -> k i p", i=CHUNKS)
            with tc.tile_pool(name="ps2", bufs=2, space="PSUM") as ps2:
                for g in range(GROUPS):
                    yb = ypool.tile([P, 8 * C], bf16)
                    for s in range(4 * g, 4 * g + 4):
                        q = s % 4
                        pA = ps2.tile([P, 2 * C], f32, tag="pA")
                        pB = ps2.tile([P, 2 * C], f32, tag="pB")
                        lt0 = ohTv[:, 2 * s, :]
                        lt1 = ohTv[:, 2 * s + 1, :]
                        nc.tensor.matmul(pA[:, 0:C], lt0, AB2[:, 0:C],
                                         start=True, stop=True)
                        nc.tensor.matmul(pA[:, C:2 * C], lt1, AB2[:, 0:C],
                                         start=True, stop=True)
                        nc.tensor.matmul(pB[:, 0:C], lt0, AB2[:, C:2 * C],
                                         start=True, stop=True)
                        nc.tensor.matmul(pB[:, C:2 * C], lt1, AB2[:, C:2 * C],
                                         start=True, stop=True)
                        ysl = yb[:, q * 2 * C:(q + 1) * 2 * C]
                        xsl = xs[g][:, q * 2 * C:(q + 1) * 2 * C]
                        a_sb = abpool.tile([P, 2 * C], bf16, tag="a_sb")
                        nc.scalar.activation(a_sb[:], pA[:], Act.Copy)
                        nc.vector.tensor_mul(ysl, xsl, a_sb[:])
                        if (s % 16) % 2 == 1 or (s % 16) == 14:
                            nc.vector.tensor_add(ysl, ysl, pB[:])
                        else:
                            b_sb = abpool.tile([P, 2 * C], bf16, tag="b_sb")
                            nc.scalar.activation(b_sb[:], pB[:], Act.Copy)
                            nc.vector.tensor_add(ysl, ysl, b_sb[:])
                    if g < GROUPS - 1:
                        nc.gpsimd.dma_start(
                            Yv[:, 8 * g:8 * g + 8, :],
                            yb[:].rearrange("p (n c) -> p n c", c=C))
                    else:
                        # split the last store so its first half overlaps
                        # the final FMAs
                        for h in range(2):
                            nc.gpsimd.dma_start(
                                Yv[:, 8 * g + 4 * h:8 * g + 4 * h + 4, :],
                                yb[:, h * 4 * C:(h + 1) * 4 * C].rearrange(
                                    "p (n c) -> p n c", c=C))

    nc.compile()
    return nc


def _get_program():
    if "nc" not in _CACHE:
        _CACHE["nc"] = _build_program()
    return _CACHE["nc"]


def _constants():
    if "cst" not in _CACHE:
        import ml_dtypes
        rep = np.tile(np.arange(D, dtype=np.float32), (P, CHUNKS))
        cst_rep = rep.astype(ml_dtypes.bfloat16)       # [P, CHUNKS*D]
        cst_col = np.arange(D, dtype=np.float32).reshape(D, 1)
        _CACHE["cst"] = (cst_rep, cst_col)
    return _CACHE["cst"]


def kernel(X, d, parameter_t, fm_mean, gamma, beta):
    from concourse.bass_utils import run_bass_kernel_spmd

    X = np.ascontiguousarray(np.asarray(X), dtype=np.float32)
    d = np.ascontiguousarray(np.asarray(d), dtype=np.int32)
    gamma = np.ascontiguousarray(np.asarray(gamma), dtype=np.float32)
    beta = np.ascontiguousarray(np.asarray(beta), dtype=np.float32)

    nc = _get_program()
    cst_rep, cst_col = _constants()
    in_maps = [
        {
            "X": X[c * SHARD:(c + 1) * SHARD],
            "d": d[c * SHARD:(c + 1) * SHARD],
            "gamma": gamma,
            "beta": beta,
            "cst_rep": cst_rep,
            "cst_col": cst_col,
        }
        for c in range(NCORES)
    ]
    res = run_bass_kernel_spmd(nc, in_maps, core_ids=list(range(NCORES)))
    out = np.concatenate([res.results[c]["Y"] for c in range(NCORES)], axis=0)
    return out.astype(np.float32, copy=False)
